# revision 62
# baseline (speedup 1.0000x reference)
"""BloomBlock on 8 TRN2 NeuronCores — 3-launch structure.

  * L1 (data-parallel over tokens): LN1 (folded into weights on host) +
    QKV projection for each core's 256 tokens (blocks i and 15-i).
  * Host: all-gather Q/K/V, regroup per head.
  * L2 (tensor-parallel over heads): each core owns 2 heads for ALL 2048
    queries. Exact-causal attention (no padded key slots): per key chunk
    c, only queries >= 128c are scored. Transposed-score layout (keys on
    partitions, queries on free dim; softmax denominator via an appended
    ones-column on V; alibi via a bias row on K matched with a ones row
    on Q). Diagonal chunks get a post-exp binary stair mask. Fused
    row-parallel dense: each core emits a partial dense output over all
    tokens from its 2 heads' context.
  * Host: reduce dense partials + residual + dense bias -> attn_out.
  * L3 (4 token groups x 2-way tensor-parallel MLP): each core runs LN2
    on its group's 512 tokens and computes fc1/gelu/fc2 for half the 4H
    features; partial fc2 outputs are reduced on host with residual2.
"""

import os
from contextlib import ExitStack

import ml_dtypes
import numpy as np

import concourse.bass as bass
import concourse.tile as tile
from concourse import bacc, mybir
from concourse.masks import make_identity

BF16 = mybir.dt.bfloat16
F32 = mybir.dt.float32
NBF = ml_dtypes.bfloat16

S, H, NH, HD = 2048, 1024, 16, 64
NCORE = 8
QB = 128          # token/key chunk size
SC = 2 * QB       # tokens per core in L1
NSLOT = 16
NHC = H // 128    # hidden chunks
EPS = 1e-5
NORM = float(np.sqrt(HD))  # 8.0 (LAYER_NUMBER = 1)


def _blocks(i):
    return (i, 15 - i)


# ----------------------------------------------------------------------------
# L1: LN1 + QKV, data-parallel over tokens (unchanged from baseline)
# ----------------------------------------------------------------------------

def build_l1():
    """Tiny launch: LN1 + transpose only (token-DP, 256 tokens/core)."""
    nc = bacc.Bacc("TRN2", target_bir_lowering=False, debug=False,
                   num_devices=NCORE)
    # x token-major packed: [p, t, f] = token 128t+p (bf16)
    x = nc.dram_tensor("x", [128, 2, H], BF16, kind="ExternalInput")
    # xhT: [t, p, 128c + j] = xhat^T[128c+p, token 128t+j]
    xhT = nc.dram_tensor("xhT", [2, 128, NHC * 128], BF16,
                         kind="ExternalOutput")

    with tile.TileContext(nc) as tc, ExitStack() as ctx:
        singles = ctx.enter_context(tc.tile_pool(name="singles", bufs=1))
        stat = ctx.enter_context(tc.tile_pool(name="stat", bufs=2))
        work = ctx.enter_context(tc.tile_pool(name="work", bufs=2))

        ident = singles.tile([128, 128], BF16)
        make_identity(nc, ident)
        epst = singles.tile([128, 1], F32)
        nc.vector.memset(epst, EPS)
        warm = singles.tile([1, 1], F32)
        nc.scalar.activation(out=warm, in_=epst[0:1, 0:1],
                             func=mybir.ActivationFunctionType.Sqrt,
                             bias=0.0, scale=1.0)
        xt = singles.tile([128, 2, H], BF16)
        nc.sync.dma_start(out=xt[:, 0, :], in_=x.ap()[:, 0, :])
        nc.sync.dma_start(out=xt[:, 1, :], in_=x.ap()[:, 1, :])
        stage = singles.tile([128, 2, NHC * 128], BF16)

        with tc.tile_pool(name="tp_ps", bufs=2, space="PSUM") as tp_ps:
            # p-state warm-up (overwritten by the real transposes)
            pw = tp_ps.tile([128, 512], F32, tag="pw", bufs=1, name="pw")
            for _ in range(20):
                nc.tensor.matmul(pw[:, 0:128], lhsT=ident[0:1, :],
                                 rhs=ident[0:1, :], start=True, stop=True)
            # both stat chains first so the DVE stream never gates t=1
            rstds = []
            for t in range(2):
                st = stat.tile([128, 2, 6], F32, tag="bnst")
                nc.vector.bn_stats(out=st[:, 0, :], in_=xt[:, t, 0:512])
                nc.vector.bn_stats(out=st[:, 1, :], in_=xt[:, t, 512:1024])
                mv = stat.tile([128, 2], F32, tag="bnmv")
                nc.vector.bn_aggr(out=mv, in_=st)
                rstd = stat.tile([128, 1], F32, tag="rstd")
                nc.scalar.activation(out=rstd, in_=mv[:, 1:2],
                                     func=mybir.ActivationFunctionType.Sqrt,
                                     bias=epst, scale=1.0)
                nc.vector.reciprocal(out=rstd, in_=rstd)
                rstds.append((mv, rstd))
            for t in range(2):
                mv, rstd = rstds[t]
                xh = work.tile([128, H], BF16, tag="xhat")
                nc.vector.tensor_scalar(out=xh, in0=xt[:, t, :],
                                        scalar1=mv[:, 0:1], scalar2=rstd,
                                        op0=mybir.AluOpType.subtract,
                                        op1=mybir.AluOpType.mult)
                tp = tp_ps.tile([128, NHC * 128], BF16, tag="tp")
                for c in range(NHC):
                    nc.tensor.transpose(tp[:, c * 128:(c + 1) * 128],
                                        xh[:, c * 128:(c + 1) * 128], ident)
                nc.vector.tensor_copy(out=stage[:, t, :], in_=tp)
                nc.sync.dma_start(out=xhT.ap()[t], in_=stage[:, t, :])
    nc.compile()
    return nc


# ----------------------------------------------------------------------------
# L2: exact-causal attention, tensor-parallel over heads (2 heads/core),
#     fused row-parallel dense partial.
# ----------------------------------------------------------------------------

def build_l2():
    """QKV (2 heads/core over all tokens) + exact-causal attention +
    row-parallel dense partial."""
    nc = bacc.Bacc("TRN2", target_bir_lowering=False, debug=False,
                   num_devices=NCORE)
    # xhat^T chunk-major (replicated input): [c, p, tok]
    xhD = nc.dram_tensor("xh", [NHC, 128, S], BF16, kind="ExternalInput")
    # lhsT weight packs, chunk c at cols [128c,128c+128): [q0|q1], [k0|k1]
    wQD = nc.dram_tensor("wQ", [128, NHC * 128], BF16, kind="ExternalInput")
    wKD = nc.dram_tensor("wK", [128, NHC * 128], BF16, kind="ExternalInput")
    # v rhs pack, chunk c: [128 h, [v0|v1] 128]
    wVD = nc.dram_tensor("wV", [128, NHC * 128], BF16, kind="ExternalInput")
    bwD = nc.dram_tensor("bw", [1, 256], BF16, kind="ExternalInput")
    bvrD = nc.dram_tensor("bvr", [1, 128], BF16, kind="ExternalInput")
    # alibi columns: [p, 16h + c] = alibi[head h, key 128c+p]
    alcD = nc.dram_tensor("alc", [128, 32], F32, kind="ExternalInput")
    binmD = nc.dram_tensor("binm", [QB, QB], BF16, kind="ExternalInput")
    dwD = nc.dram_tensor("dw", [128, H], BF16, kind="ExternalInput")
    # dense partial, token-major packed: [p, 1024*t + f] = token 128t+p
    outD = nc.dram_tensor("outD", [128, 16 * H], BF16, kind="ExternalOutput")

    with tile.TileContext(nc) as tc, ExitStack() as ctx:
        singles = ctx.enter_context(tc.tile_pool(name="singles", bufs=1))
        probs = ctx.enter_context(tc.tile_pool(name="probs", bufs=14))
        work = ctx.enter_context(tc.tile_pool(name="work", bufs=2))
        outp = ctx.enter_context(tc.tile_pool(name="outp", bufs=3))

        binm = singles.tile([QB, QB], BF16)
        dw = singles.tile([128, H], BF16)
        ctxT = singles.tile([128, S], BF16)
        qaugP = singles.tile([128, S], BF16)
        kaugP = singles.tile([128, S], BF16)
        alc_sb = singles.tile([128, 32], F32)
        bw_sb = singles.tile([1, 256], BF16)
        bv_sb = singles.tile([1, 128], BF16)
        ones_row = singles.tile([1, 1024], BF16)
        nc.vector.memset(ones_row, 1.0)
        vaug = []
        for h in range(2):
            va = singles.tile([128, 16, 65], BF16, tag=f"vaug{h}",
                              name=f"vaug{h}")
            nc.vector.memset(va, 1.0)
            vaug.append(va)
        epst = singles.tile([1, 1], F32)
        nc.vector.memset(epst, EPS)
        warm = singles.tile([1, 1], F32)
        nc.scalar.activation(out=warm, in_=epst,
                             func=mybir.ActivationFunctionType.Exp,
                             bias=0.0, scale=1.0)

        wQ = singles.tile([128, NHC * 128], BF16)
        nc.sync.dma_start(out=wQ, in_=wQD.ap())
        wK = singles.tile([128, NHC * 128], BF16)
        nc.sync.dma_start(out=wK, in_=wKD.ap())
        xh = []
        for c in range(NHC):
            xc = singles.tile([128, S], BF16, tag=f"xh{c}", name=f"xh{c}")
            nc.sync.dma_start(out=xc, in_=xhD.ap()[c])
            xh.append(xc)
        wV = singles.tile([128, NHC * 128], BF16)
        nc.sync.dma_start(out=wV, in_=wVD.ap())
        nc.sync.dma_start(out=bw_sb, in_=bwD.ap())
        nc.sync.dma_start(out=bv_sb, in_=bvrD.ap())
        nc.sync.dma_start(out=alc_sb, in_=alcD.ap())
        nc.sync.dma_start(out=binm, in_=binmD.ap())
        nc.sync.dma_start(out=dw, in_=dwD.ap())

        # ---------------- P1: Q/K projections (column halves) ------------
        # both halves accumulate interleaved; half 0 ([0:1024), all the A
        # sweep needs) drains first so attention exps start ~8us earlier
        with tc.tile_pool(name="qkv_ps", bufs=1, space="PSUM") as pqk:
            psQh = [pqk.tile([128, 1024], F32, tag=f"pq{hf}", bufs=1,
                             name=f"psQ{hf}") for hf in range(2)]
            psKh = [pqk.tile([128, 1024], F32, tag=f"pk{hf}", bufs=1,
                             name=f"psK{hf}") for hf in range(2)]
            # p-state warm-up: keep PE continuously busy from t~0.8us so the
            # real matmuls dispatch at full clock (results are overwritten by
            # the first start=True accumulation below)
            for _ in range(7):
                nc.tensor.matmul(psQh[0][:, 0:512],
                                 lhsT=ones_row[0:1, 0:128],
                                 rhs=ones_row[0:1, 0:512],
                                 start=True, stop=True)
            for c in range(NHC):
                for hf in range(2):
                    for n in range(2):
                        nc.tensor.matmul(
                            psQh[hf][:, n * 512:(n + 1) * 512],
                            lhsT=wQ[:, c * 128:(c + 1) * 128],
                            rhs=xh[c][:, hf * 1024 + n * 512:
                                       hf * 1024 + (n + 1) * 512],
                            start=(c == 0), stop=False)
                for hf in range(2):
                    for n in range(2):
                        nc.tensor.matmul(
                            psKh[hf][:, n * 512:(n + 1) * 512],
                            lhsT=wK[:, c * 128:(c + 1) * 128],
                            rhs=xh[c][:, hf * 1024 + n * 512:
                                       hf * 1024 + (n + 1) * 512],
                            start=(c == 0), stop=False)
            # bias as rank-1 updates so the drains become plain copies
            # that split across DVE and Act in parallel
            for hf in range(2):
                for n in range(2):
                    nc.tensor.matmul(
                        psQh[hf][:, n * 512:(n + 1) * 512],
                        lhsT=bw_sb[0:1, 0:128],
                        rhs=ones_row[:, 0:512],
                        start=False, stop=True)
                    nc.tensor.matmul(
                        psKh[hf][:, n * 512:(n + 1) * 512],
                        lhsT=bw_sb[0:1, 128:256],
                        rhs=ones_row[:, 0:512],
                        start=False, stop=True)
            for hf in range(2):
                lo = hf * 1024
                nc.vector.tensor_copy(out=qaugP[:, lo:lo + 1024],
                                      in_=psQh[hf])
                nc.scalar.activation(out=kaugP[:, lo:lo + 1024],
                                     in_=psKh[hf],
                                     func=mybir.ActivationFunctionType.Copy,
                                     bias=0.0, scale=1.0)

        # ---------------- P2: A-sweep scores/exp + V, interleaved ----------
        # A-sweep probs are saved to dedicated tiles; their ctx matmuls are
        # deferred to P3 where they fill PE slack under the B-sweep.
        probsA = ctx.enter_context(tc.tile_pool(name="probsA", bufs=1))
        pA = {}
        with tc.tile_pool(name="ps", bufs=1, space="PSUM") as pp:
            vps = [pp.tile([128, 8, 128], F32, tag="vp", bufs=2,
                           name=f"vp{b}") for b in range(2)]
            for wi, (h, c) in enumerate([(h, c) for c in range(8)
                                         for h in range(2)]):
                base = 128 * c
                n = 1024 - base
                p0 = 64 * h
                ps = pp.tile([128, 1024], F32, tag="sc", bufs=2, name="psA")
                for s in range(0, n, 512):
                    sl = min(512, n - s)
                    nc.tensor.matmul(
                        ps[:, s:s + sl],
                        lhsT=kaugP[p0:p0 + 64, base:base + 128],
                        rhs=qaugP[p0:p0 + 64, base + s:base + s + sl],
                        start=True, stop=True)
                pb = probsA.tile([128, 1024], BF16, tag=f"pA{wi}",
                                 name=f"pA{wi}")
                col = 16 * h + c
                nc.scalar.activation(out=pb[:, 0:n], in_=ps[:, 0:n],
                                     func=mybir.ActivationFunctionType.Exp,
                                     bias=alc_sb[:, col:col + 1], scale=1.0)
                nc.vector.tensor_mul(out=pb[:, 0:128], in0=pb[:, 0:128],
                                     in1=binm)
                pA[(h, c)] = pb
                # one V token-tile per window (PE filler under Act exps)
                t = wi
                b, ti = t // 8, t % 8
                for cc in range(NHC):
                    nc.tensor.matmul(
                        vps[b][:, ti, :],
                        lhsT=xh[cc][:, t * 128:(t + 1) * 128],
                        rhs=wV[:, cc * 128:(cc + 1) * 128],
                        start=(cc == 0), stop=False)
                nc.tensor.matmul(vps[b][:, ti, :], lhsT=ones_row[:, 0:128],
                                 rhs=bv_sb, start=False, stop=True)
                if ti == 7:
                    for h2 in range(2):
                        nc.vector.tensor_copy(
                            out=vaug[h2][:, 8 * b:8 * (b + 1), 0:64],
                            in_=vps[b][:, :, 64 * h2:64 * h2 + 64])

            # ------------- P3: B-sweep + deferred A-ctx + dense -------------

            def _dense_tile(t, use_act=False):
                psd = pp.tile([128, 1024], F32, tag="sc", bufs=2, name="psd")
                for n in range(2):
                    nc.tensor.matmul(
                        psd[:, n * 512:(n + 1) * 512],
                        lhsT=ctxT[:, t * 128:(t + 1) * 128],
                        rhs=dw[:, n * 512:(n + 1) * 512],
                        start=True, stop=True)
                stg = outp.tile([128, 1024], BF16, tag="og")
                if use_act:
                    nc.scalar.activation(
                        out=stg, in_=psd,
                        func=mybir.ActivationFunctionType.Copy,
                        bias=0.0, scale=1.0)
                else:
                    nc.vector.tensor_copy(out=stg, in_=psd)
                nc.sync.dma_start(out=outD.ap()[:, t * H:(t + 1) * H],
                                  in_=stg)

            def _scores(h, c, qa, qb):
                base = 128 * c
                n = qb - qa
                p0 = 64 * h
                ps = pp.tile([128, 1024], F32, tag="sc", bufs=2, name="ps")
                for s in range(0, n, 512):
                    sl = min(512, n - s)
                    nc.tensor.matmul(
                        ps[:, s:s + sl],
                        lhsT=kaugP[p0:p0 + 64, base:base + 128],
                        rhs=qaugP[p0:p0 + 64, qa + s:qa + s + sl],
                        start=True, stop=True)
                pb = probs.tile([128, 1024], BF16, tag="pb")
                col = 16 * h + c
                nc.scalar.activation(out=pb[:, 0:n], in_=ps[:, 0:n],
                                     func=mybir.ActivationFunctionType.Exp,
                                     bias=alc_sb[:, col:col + 1], scale=1.0)
                if qa == base:
                    nc.vector.tensor_mul(out=pb[:, 0:128], in0=pb[:, 0:128],
                                         in1=binm)
                return pb

            def _ctx(h, pc, c, qa, qb, pb):
                n = qb - qa
                slices = []
                if qa == 128 * c:
                    slices.append((0, 128, True))
                    p = 128
                else:
                    p = 0
                while p < n:
                    nxt = min(n, ((qa + p) // 512 + 1) * 512 - qa)
                    slices.append((p, nxt - p, False))
                    p = nxt
                off = qa if qa < 1024 else qa - 1024
                for (s, sl, stp) in slices:
                    o = off + s
                    nc.tensor.matmul(pc[0:65, o // 128:(o + sl) // 128, :],
                                     lhsT=vaug[h][:, c, 0:65],
                                     rhs=pb[:, s:s + sl],
                                     start=(c == 0), stop=stp,
                                     skip_group_check=True)

            def _normalize(pc, src0, h, dst0, n):
                a, b2 = src0 // 128, (src0 + n) // 128
                rec = work.tile([1, 1024], F32, tag="rec")
                nc.vector.reciprocal(out=rec[:, 0:n], in_=pc[64:65, a:b2, :])
                recb = work.tile([64, 1024], F32, tag="recb")
                nc.gpsimd.partition_broadcast(recb[:, 0:n], rec[0:1, 0:n],
                                              channels=64)
                nc.vector.tensor_mul(
                    out=ctxT[64 * h:64 * h + 64, dst0:dst0 + n],
                    in0=pc[0:64, a:b2, :], in1=recb[:, 0:n])

            pcA = [pp.tile([128, 8, 128], F32, tag="vp", bufs=2,
                           name=f"pcA{h}") for h in range(2)]
            pcB = [pp.tile([128, 8, 128], F32, tag="vp", bufs=2,
                           name=f"pcB{h}") for h in range(2)]
            winsB = [(h, c, (1024 if c < 8 else 128 * c), 2048)
                     for c in range(16) for h in range(2)]
            pendq = []
            actx = [(h, c) for c in range(8) for h in range(2)]
            nd = 0
            avail = 0
            na = 0
            for wi, (h, c, qa, qb) in enumerate(winsB):
                pb = _scores(h, c, qa, qb)
                pendq.append((h, pcB[h], c, qa, qb, pb))
                # A-ctx fillers: two per window over the first 8 windows
                if wi < 8:
                    for _ in range(2):
                        ah, ac = actx[na]
                        _ctx(ah, pcA[ah], ac, 128 * ac, 1024, pA[(ah, ac)])
                        na += 1
                # B-ctx: deferred until pcA is consumed (its psum banks are
                # reused by pcB); catch up 3 per window
                if wi >= 12:
                    for _ in range(4):
                        if len(pendq) > 1:
                            _ctx(*pendq.pop(0))
                if wi == 8:
                    _normalize(pcA[0], 0, 0, 0, 1024)
                    _normalize(pcA[1], 0, 1, 0, 1024)
                    avail = 8
                if wi == 24:
                    _normalize(pcB[0], 0, 0, 1024, 512)
                    _normalize(pcB[1], 0, 1, 1024, 512)
                    avail = 12
                if wi in (26, 28, 30):
                    k = (wi - 2) // 2 - 8   # 4, 5, 6
                    _normalize(pcB[0], 128 * k, 0, 1024 + 128 * k, 128)
                    _normalize(pcB[1], 128 * k, 1, 1024 + 128 * k, 128)
                    avail = 9 + k
                if wi >= 10 and nd < avail:
                    _dense_tile(nd, use_act=(nd >= 6))
                    nd += 1
            while pendq:
                _ctx(*pendq.pop(0))
            if nd < 15:
                _dense_tile(nd, use_act=True)
                nd += 1
            _normalize(pcB[0], 896, 0, 1920, 128)
            _normalize(pcB[1], 896, 1, 1920, 128)
            while nd < 16:
                _dense_tile(nd, use_act=(nd % 2 == 1))
                nd += 1
    nc.compile()
    return nc


# ----------------------------------------------------------------------------
# L3: LN2 + MLP, 4 token groups x 2-way tensor-parallel over 4H
# ----------------------------------------------------------------------------

def build_l3():
    nc = bacc.Bacc("TRN2", target_bir_lowering=False, debug=False,
                   num_devices=NCORE)
    # xres: [p, 1024t + f] = attn_out token 128t+p (512 tokens per core)
    xresD = nc.dram_tensor("xres", [128, 4 * H], BF16, kind="ExternalInput")
    # f1T: m-major packing: [p, 1024*m + 128*c + j] = fc1_eff.T[128c+p, 2048*half + 128m + j]
    f1TD = nc.dram_tensor("f1T", [4, 128, 4 * H], BF16, kind="ExternalInput")
    b1D = nc.dram_tensor("b1c", [128, 16], F32, kind="ExternalInput")
    # f2T: chunk m at cols [1024m:1024(m+1)): fc2_w.T[2048*half+128m+p, f]
    f2TD = nc.dram_tensor("f2T", [4, 128, 4 * H], BF16, kind="ExternalInput")
    outP = nc.dram_tensor("outP", [128, 4 * H], BF16, kind="ExternalOutput")

    with tile.TileContext(nc) as tc, ExitStack() as ctx:
        singles = ctx.enter_context(tc.tile_pool(name="singles", bufs=1))
        stat = ctx.enter_context(tc.tile_pool(name="stat", bufs=2))
        work = ctx.enter_context(tc.tile_pool(name="work", bufs=2))
        hp = ctx.enter_context(tc.tile_pool(name="hp", bufs=1))
        outp = ctx.enter_context(tc.tile_pool(name="outp", bufs=2))

        ident = singles.tile([128, 128], BF16)
        make_identity(nc, ident)
        epst = singles.tile([128, 1], F32)
        nc.vector.memset(epst, EPS)
        warm = singles.tile([1, 1], F32)
        nc.scalar.activation(out=warm, in_=epst[0:1, 0:1],
                             func=mybir.ActivationFunctionType.Sqrt,
                             bias=0.0, scale=1.0)

        xres = singles.tile([128, 4, H], BF16)
        b1c = singles.tile([128, 16], F32)
        f1 = [singles.tile([128, 4 * H], BF16, tag=f"f1{g}", name=f"f1{g}")
              for g in range(4)]
        f2 = [singles.tile([128, 4 * H], BF16, tag=f"f2{g}", name=f"f2{g}")
              for g in range(4)]
        # interleave: first two xres tiles (gate LN2 for fc1's first half),
        # then the first fc1 group, then the rest
        nc.sync.dma_start(out=xres[:, 0, :], in_=xresD.ap()[:, 0:H])
        nc.sync.dma_start(out=xres[:, 1, :], in_=xresD.ap()[:, H:2 * H])
        nc.sync.dma_start(out=b1c, in_=b1D.ap())
        nc.sync.dma_start(out=f1[0], in_=f1TD.ap()[0])
        nc.sync.dma_start(out=xres[:, 2, :], in_=xresD.ap()[:, 2 * H:3 * H])
        nc.sync.dma_start(out=xres[:, 3, :], in_=xresD.ap()[:, 3 * H:4 * H])
        for g in range(1, 4):
            nc.sync.dma_start(out=f1[g], in_=f1TD.ap()[g])
        for g in range(4):
            nc.sync.dma_start(out=f2[g], in_=f2TD.ap()[g])

        # xh2T: [p, c, 128t+j] = xhat^T chunk c
        xh2T = singles.tile([128, NHC, 512], BF16)

        ones_pe = singles.tile([1, 512], BF16)
        nc.vector.memset(ones_pe, 1.0)
        with tc.tile_pool(name="ps", bufs=1, space="PSUM") as pp:
            pwarm = pp.tile([128, 512], F32, tag="f1", bufs=3, name="pwarm")
            for _ in range(9):
                nc.tensor.matmul(pwarm, lhsT=ones_pe[0:1, 0:128],
                                 rhs=ones_pe[0:1, 0:512],
                                 start=True, stop=True)
            for t in range(4):
                st = stat.tile([128, 2, 6], F32, tag="bnst")
                nc.vector.bn_stats(out=st[:, 0, :], in_=xres[:, t, 0:512])
                nc.vector.bn_stats(out=st[:, 1, :], in_=xres[:, t, 512:1024])
                mv = stat.tile([128, 2], F32, tag="bnmv")
                nc.vector.bn_aggr(out=mv, in_=st)
                rstd = stat.tile([128, 1], F32, tag="rstd")
                nc.scalar.activation(out=rstd, in_=mv[:, 1:2],
                                     func=mybir.ActivationFunctionType.Sqrt,
                                     bias=epst, scale=1.0)
                nc.vector.reciprocal(out=rstd, in_=rstd)
                xh = work.tile([128, H], BF16, tag="xhat")
                nc.vector.tensor_scalar(out=xh, in0=xres[:, t, :],
                                        scalar1=mv[:, 0:1], scalar2=rstd,
                                        op0=mybir.AluOpType.subtract,
                                        op1=mybir.AluOpType.mult)
                tp = pp.tile([128, 1024], BF16, tag="tp", bufs=1)
                for c in range(NHC):
                    nc.tensor.transpose(tp[:, c * 128:(c + 1) * 128],
                                        xh[:, c * 128:(c + 1) * 128], ident)
                nc.vector.tensor_copy(out=xh2T[:, :, t * 128:(t + 1) * 128],
                                      in_=tp)

            hts = {}

            def _fc1(m):
                ps = pp.tile([128, 512], F32, tag="f1", bufs=3, name="psf1")
                # token-quarters: the first fc1 only waits on LN2 of tile 0
                for tg in range(4):
                    for c in range(NHC):
                        nc.tensor.matmul(
                            ps[:, tg * 128:(tg + 1) * 128],
                            lhsT=f1[m // 4][:, (m % 4) * 1024 + c * 128:
                                            (m % 4) * 1024 + (c + 1) * 128],
                            rhs=xh2T[:, c, tg * 128:(tg + 1) * 128],
                            start=(c == 0), stop=(c == NHC - 1))
                ht = hp.tile([128, 512], BF16, tag=f"h{m}")
                if os.environ.get("BLOOM_SIM"):
                    u = work.tile([128, 512], F32, tag="gelu_u")
                    nc.vector.tensor_scalar_add(out=u, in0=ps,
                                                scalar1=b1c[:, m:m + 1])
                    s2 = work.tile([128, 512], F32, tag="gelu_s")
                    nc.vector.tensor_mul(out=s2, in0=u, in1=u)
                    nc.vector.tensor_scalar(out=s2, in0=s2,
                                            scalar1=0.035677408145115,
                                            scalar2=0.7978845608028654,
                                            op0=mybir.AluOpType.mult,
                                            op1=mybir.AluOpType.add)
                    nc.vector.tensor_mul(out=s2, in0=s2, in1=u)
                    nc.scalar.activation(out=s2, in_=s2,
                                         func=mybir.ActivationFunctionType.Tanh,
                                         bias=0.0, scale=1.0)
                    nc.vector.tensor_scalar(out=s2, in0=s2, scalar1=1.0,
                                            scalar2=0.5,
                                            op0=mybir.AluOpType.add,
                                            op1=mybir.AluOpType.mult)
                    nc.vector.tensor_mul(out=ht, in0=s2, in1=u)
                else:
                    nc.scalar.activation(
                        out=ht, in_=ps,
                        func=mybir.ActivationFunctionType.Gelu_apprx_tanh,
                        bias=b1c[:, m:m + 1], scale=1.0)
                hts[m] = ht

            def _fc2(psf2, m, tpair):
                ht = hts[m]
                for ti, t in enumerate(tpair):
                    for n in range(2):
                        nc.tensor.matmul(
                            psf2[ti][:, n * 512:(n + 1) * 512],
                            lhsT=ht[:, t * 128:(t + 1) * 128],
                            rhs=f2[m // 4][:, (m % 4) * 1024 + n * 512:
                                           (m % 4) * 1024 + (n + 1) * 512],
                            start=(m == 0), stop=(m == 15))

            def _drain(psf2, tpair):
                for ti, t in enumerate(tpair):
                    stg = outp.tile([128, 1024], BF16, tag="og")
                    if t % 2 == 0:
                        nc.vector.tensor_copy(out=stg, in_=psf2[ti])
                    else:
                        nc.scalar.activation(
                            out=stg, in_=psf2[ti],
                            func=mybir.ActivationFunctionType.Copy,
                            bias=0.0, scale=1.0)
                    nc.sync.dma_start(out=outP.ap()[:, t * H:(t + 1) * H],
                                      in_=stg)

            # pass 1: fc1 all m, fc2 into token tiles 0,1 (staggered)
            psf2 = [pp.tile([128, 1024], F32, tag=f"f2_{t}", bufs=1,
                            name=f"psf2_{t}")
                    for t in range(2)]
            _fc1(0)
            for m in range(16):
                if m + 1 < 16:
                    _fc1(m + 1)
                _fc2(psf2, m, (0, 1))
            _drain(psf2, (0, 1))
            # passes 2/3: fc2 for token tiles 2 then 3 (staggered drains)
            psf2b = [pp.tile([128, 1024], F32, tag="f2_0", bufs=1,
                             name="psf2b")]
            for m in range(16):
                _fc2(psf2b, m, (2,))
            _drain(psf2b, (2,))
            psf2c = [pp.tile([128, 1024], F32, tag="f2_1", bufs=1,
                             name="psf2c")]
            for m in range(16):
                _fc2(psf2c, m, (3,))
            _drain(psf2c, (3,))
    nc.compile()
    return nc


# ----------------------------------------------------------------------------
# host orchestration
# ----------------------------------------------------------------------------

_NC_CACHE = {}
_BUILDERS = {"l1": build_l1, "l2": build_l2, "l3": build_l3}


def _get_nc(name):
    if name not in _NC_CACHE:
        _NC_CACHE[name] = _BUILDERS[name]()
    return _NC_CACHE[name]


def _run(nc, in_maps):
    if os.environ.get("BLOOM_SIM"):
        from concourse.bass_interp import CoreSim
        results = []
        for m in in_maps:
            sim = CoreSim(nc, trace=False)
            for k, v in m.items():
                sim.tensor(k)[:] = v
            sim.simulate(check_with_hw=False)
            outs = {}
            for alloc in nc.m.functions[0].allocations:
                if getattr(alloc, "kind", None) == "ExternalOutput":
                    nm = alloc.memorylocations[0].name
                    outs[nm] = np.array(sim.tensor(nm))
            results.append(outs)
        return results
    from concourse.bass_utils import run_bass_kernel_spmd
    res = run_bass_kernel_spmd(nc, in_maps, core_ids=list(range(NCORE)))
    return res.results


def _prep_weights(ln1_g, ln1_b, qkv_w, qkv_b, dense_w, dense_b,
                  ln2_g, ln2_b, fc1_w, fc1_b, fc2_w, fc2_b):
    qkv_w = np.asarray(qkv_w, np.float32)
    qkv_b = np.asarray(qkv_b, np.float32)
    w_eff = qkv_w * np.asarray(ln1_g, np.float32)[None, :]
    b_eff = qkv_b + qkv_w @ np.asarray(ln1_b, np.float32)
    w3 = w_eff.reshape(NH, 3 * HD, H)
    b3 = b_eff.reshape(NH, 3 * HD)
    wq = w3[:, :HD, :] / NORM
    wk = w3[:, HD:2 * HD, :]
    wv = w3[:, 2 * HD:, :]
    bq = b3[:, :HD] / NORM
    bk = b3[:, HD:2 * HD]
    bv = b3[:, 2 * HD:]

    def pack_lhsT(w_pair):
        """[128 f, 1024 h] -> [p, 128c + f] = w_pair[f, 128c + p]"""
        return np.ascontiguousarray(
            w_pair.T.reshape(NHC, 128, 128).transpose(1, 0, 2)
            .reshape(128, NHC * 128)).astype(NBF)

    wQ_i, wK_i, wV_i, bcol_i, bvr_i = [], [], [], [], []
    for i in range(NCORE):
        h0, h1 = 2 * i, 2 * i + 1
        wQ_i.append(pack_lhsT(np.concatenate([wq[h0], wq[h1]], 0)))
        wK_i.append(pack_lhsT(np.concatenate([wk[h0], wk[h1]], 0)))
        wV_i.append(pack_lhsT(np.concatenate([wv[h0], wv[h1]], 0)))
        bcol_i.append(np.concatenate(
            [bq[h0], bq[h1], bk[h0], bk[h1]]).reshape(1, 256).astype(NBF))
        bvr_i.append(np.concatenate([bv[h0], bv[h1]])
                     .reshape(1, 128).astype(NBF))

    dwT = np.ascontiguousarray(np.asarray(dense_w, np.float32).T).astype(NBF)
    db_r = np.asarray(dense_b, np.float32).reshape(1, H)

    f1_eff = np.asarray(fc1_w, np.float32) * np.asarray(ln2_g, np.float32)[None, :]
    b1_eff = np.asarray(fc1_b, np.float32) + np.asarray(fc1_w, np.float32) @ np.asarray(ln2_b, np.float32)
    fc1T = np.ascontiguousarray(f1_eff.T)                           # [H, 4H]
    fc2T = np.ascontiguousarray(np.asarray(fc2_w, np.float32).T)    # [4H, H]
    b2_r = np.asarray(fc2_b, np.float32).reshape(1, H)

    # L3 packings, per half
    f1T_half, b1_half, f2T_half = [], [], []
    for half in range(2):
        cols = slice(half * 2 * H, (half + 1) * 2 * H)
        f1h = fc1T[:, cols]                                         # [1024, 2048]
        # f1TD[g, p, 1024*(m%4) + 128c + j] = f1h[128c + p, 128m + j]
        a = (f1h.reshape(NHC, 128, 16, 128)      # [c, p, m, j]
             .transpose(2, 1, 0, 3)              # [m, p, c, j]
             .reshape(4, 4, 128, NHC * 128)      # [g, m%4, p, c*j]
             .transpose(0, 2, 1, 3)              # [g, p, m%4, c*j]
             .reshape(4, 128, 4 * H))
        f1T_half.append(np.ascontiguousarray(a.astype(NBF)))
        b1h = b1_eff[half * 2 * H:(half + 1) * 2 * H]
        b1_half.append(np.ascontiguousarray(
            b1h.reshape(16, 128).T).astype(np.float32))
        f2h = fc2T[half * 2 * H:(half + 1) * 2 * H, :]              # [2048, 1024]
        # f2TD[g, p, 1024*(m%4) + f] = f2h[128m + p, f]
        b = (f2h.reshape(4, 4, 128, H)           # [g, m%4, p, f]
             .transpose(0, 2, 1, 3)              # [g, p, m%4, f]
             .reshape(4, 128, 4 * H))
        f2T_half.append(np.ascontiguousarray(b.astype(NBF)))
    return dict(wQ=wQ_i, wK=wK_i, wV=wV_i, bcol=bcol_i, bvr=bvr_i, db=db_r,
                dwT=dwT, f1T_half=f1T_half, b1_half=b1_half,
                f2T_half=f2T_half, b2=b2_r)


def _tri_mask():
    k = np.arange(QB)[:, None]
    q = np.arange(QB)[None, :]
    return np.where(k <= q, 1.0, 0.0).astype(NBF)   # [k, q] allowed k<=q


def kernel(hidden_states, attention_mask, alibi,
           ln1_g, ln1_b, qkv_w, qkv_b, dense_w, dense_b,
           ln2_g, ln2_b, fc1_w, fc1_b, fc2_w, fc2_b):
    X = np.asarray(hidden_states, np.float32).reshape(S, H)
    alibi_np = np.asarray(alibi, np.float32).reshape(NH, S)
    W = _prep_weights(ln1_g, ln1_b, qkv_w, qkv_b, dense_w, dense_b,
                      ln2_g, ln2_b, fc1_w, fc1_b, fc2_w, fc2_b)

    # ---------------- L1: LN1 + transpose ----------------
    nc1 = _get_nc("l1")
    in1 = []
    for i in range(NCORE):
        a, b = _blocks(i)
        xi = np.concatenate([X[a * QB:(a + 1) * QB], X[b * QB:(b + 1) * QB]], 0)
        xp = np.ascontiguousarray(
            xi.reshape(2, 128, H).transpose(1, 0, 2)).astype(NBF)
        in1.append(dict(x=xp))
    r1 = _run(nc1, in1)

    # ---------------- host gather: xhat^T chunk-major ----------------
    xhG = np.zeros((NHC, 128, S), NBF)
    for i in range(NCORE):
        a, b = _blocks(i)
        r = r1[i]["xhT"].reshape(2, 128, NHC, 128)
        xhG[:, :, a * QB:(a + 1) * QB] = r[0].transpose(1, 0, 2)
        xhG[:, :, b * QB:(b + 1) * QB] = r[1].transpose(1, 0, 2)
    xhG = np.ascontiguousarray(xhG)

    # ---------------- L2: QKV + attention + dense partial ----------------
    binm = _tri_mask()
    dwT = W["dwT"]
    nc2 = _get_nc("l2")
    in2 = []
    for i in range(NCORE):
        alc = np.zeros((128, 32), np.float32)
        for j in range(2):
            alc[:, 16 * j:16 * (j + 1)] = \
                alibi_np[2 * i + j].reshape(16, 128).T
        dwi = np.ascontiguousarray(dwT[i * 128:(i + 1) * 128, :]).astype(NBF)
        in2.append(dict(xh=xhG, wQ=W["wQ"][i], wK=W["wK"][i], wV=W["wV"][i],
                        bw=W["bcol"][i], bvr=W["bvr"][i], alc=alc,
                        binm=binm, dw=dwi))
    r2 = _run(nc2, in2)

    # host reduce: attn_out = sum of dense partials + residual + dense bias
    attn_out = X + W["db"]
    for i in range(NCORE):
        attn_out = attn_out + r2[i]["outD"].astype(np.float32) \
            .reshape(128, 16, H).transpose(1, 0, 2).reshape(S, H)

    # ---------------- L3 ----------------
    nc3 = _get_nc("l3")
    in3 = []
    attn_bf = attn_out.astype(NBF)
    for i in range(NCORE):
        g, half = i // 2, i % 2
        xg = attn_bf[512 * g:512 * (g + 1)]        # [512, H]
        xres = np.ascontiguousarray(
            xg.reshape(4, 128, H).transpose(1, 0, 2).reshape(128, 4 * H))
        in3.append(dict(xres=xres, f1T=W["f1T_half"][half],
                        b1c=W["b1_half"][half], f2T=W["f2T_half"][half]))
    r3 = _run(nc3, in3)

    out = np.empty((S, H), np.float32)
    for g in range(4):
        p = r3[2 * g]["outP"].astype(np.float32) + \
            r3[2 * g + 1]["outP"].astype(np.float32)
        out[512 * g:512 * (g + 1)] = \
            p.reshape(128, 4, H).transpose(1, 0, 2).reshape(512, H) \
            + attn_out[512 * g:512 * (g + 1)] + W["b2"]
    return out.reshape(1, S, H)


# revision 63
# speedup vs baseline: 1.0004x; 1.0004x over previous
"""BloomBlock on 8 TRN2 NeuronCores — 3-launch structure.

  * L1 (data-parallel over tokens): LN1 (folded into weights on host) +
    QKV projection for each core's 256 tokens (blocks i and 15-i).
  * Host: all-gather Q/K/V, regroup per head.
  * L2 (tensor-parallel over heads): each core owns 2 heads for ALL 2048
    queries. Exact-causal attention (no padded key slots): per key chunk
    c, only queries >= 128c are scored. Transposed-score layout (keys on
    partitions, queries on free dim; softmax denominator via an appended
    ones-column on V; alibi via a bias row on K matched with a ones row
    on Q). Diagonal chunks get a post-exp binary stair mask. Fused
    row-parallel dense: each core emits a partial dense output over all
    tokens from its 2 heads' context.
  * Host: reduce dense partials + residual + dense bias -> attn_out.
  * L3 (4 token groups x 2-way tensor-parallel MLP): each core runs LN2
    on its group's 512 tokens and computes fc1/gelu/fc2 for half the 4H
    features; partial fc2 outputs are reduced on host with residual2.
"""

import os
from contextlib import ExitStack

import ml_dtypes
import numpy as np

import concourse.bass as bass
import concourse.tile as tile
from concourse import bacc, mybir
from concourse.masks import make_identity

BF16 = mybir.dt.bfloat16
F32 = mybir.dt.float32
NBF = ml_dtypes.bfloat16

S, H, NH, HD = 2048, 1024, 16, 64
NCORE = 8
QB = 128          # token/key chunk size
SC = 2 * QB       # tokens per core in L1
NSLOT = 16
NHC = H // 128    # hidden chunks
EPS = 1e-5
NORM = float(np.sqrt(HD))  # 8.0 (LAYER_NUMBER = 1)


def _blocks(i):
    return (i, 15 - i)


# ----------------------------------------------------------------------------
# L1: LN1 + QKV, data-parallel over tokens (unchanged from baseline)
# ----------------------------------------------------------------------------

def build_l1():
    """Tiny launch: LN1 + transpose only (token-DP, 256 tokens/core)."""
    nc = bacc.Bacc("TRN2", target_bir_lowering=False, debug=False,
                   num_devices=NCORE)
    # x token-major packed: [p, t, f] = token 128t+p (bf16)
    x = nc.dram_tensor("x", [128, 2, H], BF16, kind="ExternalInput")
    # xhT: [t, p, 128c + j] = xhat^T[128c+p, token 128t+j]
    xhT = nc.dram_tensor("xhT", [2, 128, NHC * 128], BF16,
                         kind="ExternalOutput")

    with tile.TileContext(nc) as tc, ExitStack() as ctx:
        singles = ctx.enter_context(tc.tile_pool(name="singles", bufs=1))
        stat = ctx.enter_context(tc.tile_pool(name="stat", bufs=2))
        work = ctx.enter_context(tc.tile_pool(name="work", bufs=2))

        ident = singles.tile([128, 128], BF16)
        make_identity(nc, ident)
        epst = singles.tile([128, 1], F32)
        nc.vector.memset(epst, EPS)
        warm = singles.tile([1, 1], F32)
        nc.scalar.activation(out=warm, in_=epst[0:1, 0:1],
                             func=mybir.ActivationFunctionType.Sqrt,
                             bias=0.0, scale=1.0)
        xt = singles.tile([128, 2, H], BF16)
        nc.sync.dma_start(out=xt[:, 0, :], in_=x.ap()[:, 0, :])
        nc.sync.dma_start(out=xt[:, 1, :], in_=x.ap()[:, 1, :])
        stage = singles.tile([128, 2, NHC * 128], BF16)

        with tc.tile_pool(name="tp_ps", bufs=2, space="PSUM") as tp_ps:
            # p-state warm-up (overwritten by the real transposes)
            pw = tp_ps.tile([128, 512], F32, tag="pw", bufs=1, name="pw")
            for _ in range(20):
                nc.tensor.matmul(pw[:, 0:128], lhsT=ident[0:1, :],
                                 rhs=ident[0:1, :], start=True, stop=True)
            # both stat chains first so the DVE stream never gates t=1
            rstds = []
            for t in range(2):
                st = stat.tile([128, 2, 6], F32, tag="bnst")
                nc.vector.bn_stats(out=st[:, 0, :], in_=xt[:, t, 0:512])
                nc.vector.bn_stats(out=st[:, 1, :], in_=xt[:, t, 512:1024])
                mv = stat.tile([128, 2], F32, tag="bnmv")
                nc.vector.bn_aggr(out=mv, in_=st)
                rstd = stat.tile([128, 1], F32, tag="rstd")
                nc.scalar.activation(out=rstd, in_=mv[:, 1:2],
                                     func=mybir.ActivationFunctionType.Sqrt,
                                     bias=epst, scale=1.0)
                nc.vector.reciprocal(out=rstd, in_=rstd)
                rstds.append((mv, rstd))
            for t in range(2):
                mv, rstd = rstds[t]
                xh = work.tile([128, H], BF16, tag="xhat")
                nc.vector.tensor_scalar(out=xh, in0=xt[:, t, :],
                                        scalar1=mv[:, 0:1], scalar2=rstd,
                                        op0=mybir.AluOpType.subtract,
                                        op1=mybir.AluOpType.mult)
                tp = tp_ps.tile([128, NHC * 128], BF16, tag="tp")
                for c in range(NHC):
                    nc.tensor.transpose(tp[:, c * 128:(c + 1) * 128],
                                        xh[:, c * 128:(c + 1) * 128], ident)
                nc.vector.tensor_copy(out=stage[:, t, :], in_=tp)
                nc.sync.dma_start(out=xhT.ap()[t], in_=stage[:, t, :])
    nc.compile()
    return nc


# ----------------------------------------------------------------------------
# L2: exact-causal attention, tensor-parallel over heads (2 heads/core),
#     fused row-parallel dense partial.
# ----------------------------------------------------------------------------

def build_l2():
    """QKV (2 heads/core over all tokens) + exact-causal attention +
    row-parallel dense partial."""
    nc = bacc.Bacc("TRN2", target_bir_lowering=False, debug=False,
                   num_devices=NCORE)
    # xhat^T chunk-major (replicated input): [c, p, tok]
    xhD = nc.dram_tensor("xh", [NHC, 128, S], BF16, kind="ExternalInput")
    # lhsT weight packs, chunk c at cols [128c,128c+128): [q0|q1], [k0|k1]
    wQD = nc.dram_tensor("wQ", [128, NHC * 128], BF16, kind="ExternalInput")
    wKD = nc.dram_tensor("wK", [128, NHC * 128], BF16, kind="ExternalInput")
    # v rhs pack, chunk c: [128 h, [v0|v1] 128]
    wVD = nc.dram_tensor("wV", [128, NHC * 128], BF16, kind="ExternalInput")
    bwD = nc.dram_tensor("bw", [1, 256], BF16, kind="ExternalInput")
    bvrD = nc.dram_tensor("bvr", [1, 128], BF16, kind="ExternalInput")
    # alibi columns: [p, 16h + c] = alibi[head h, key 128c+p]
    alcD = nc.dram_tensor("alc", [128, 32], F32, kind="ExternalInput")
    binmD = nc.dram_tensor("binm", [QB, QB], BF16, kind="ExternalInput")
    dwD = nc.dram_tensor("dw", [128, H], BF16, kind="ExternalInput")
    # dense partial, token-major packed: [p, 1024*t + f] = token 128t+p
    outD = nc.dram_tensor("outD", [128, 16 * H], BF16, kind="ExternalOutput")

    with tile.TileContext(nc) as tc, ExitStack() as ctx:
        singles = ctx.enter_context(tc.tile_pool(name="singles", bufs=1))
        probs = ctx.enter_context(tc.tile_pool(name="probs", bufs=14))
        work = ctx.enter_context(tc.tile_pool(name="work", bufs=3))
        outp = ctx.enter_context(tc.tile_pool(name="outp", bufs=4))

        binm = singles.tile([QB, QB], BF16)
        dw = singles.tile([128, H], BF16)
        ctxT = singles.tile([128, S], BF16)
        qaugP = singles.tile([128, S], BF16)
        kaugP = singles.tile([128, S], BF16)
        alc_sb = singles.tile([128, 32], F32)
        bw_sb = singles.tile([1, 256], BF16)
        bv_sb = singles.tile([1, 128], BF16)
        ones_row = singles.tile([1, 1024], BF16)
        nc.vector.memset(ones_row, 1.0)
        vaug = []
        for h in range(2):
            va = singles.tile([128, 16, 65], BF16, tag=f"vaug{h}",
                              name=f"vaug{h}")
            nc.vector.memset(va, 1.0)
            vaug.append(va)
        epst = singles.tile([1, 1], F32)
        nc.vector.memset(epst, EPS)
        warm = singles.tile([1, 1], F32)
        nc.scalar.activation(out=warm, in_=epst,
                             func=mybir.ActivationFunctionType.Exp,
                             bias=0.0, scale=1.0)

        wQ = singles.tile([128, NHC * 128], BF16)
        nc.sync.dma_start(out=wQ, in_=wQD.ap())
        wK = singles.tile([128, NHC * 128], BF16)
        nc.sync.dma_start(out=wK, in_=wKD.ap())
        xh = []
        for c in range(NHC):
            xc = singles.tile([128, S], BF16, tag=f"xh{c}", name=f"xh{c}")
            nc.sync.dma_start(out=xc, in_=xhD.ap()[c])
            xh.append(xc)
        wV = singles.tile([128, NHC * 128], BF16)
        nc.sync.dma_start(out=wV, in_=wVD.ap())
        nc.sync.dma_start(out=bw_sb, in_=bwD.ap())
        nc.sync.dma_start(out=bv_sb, in_=bvrD.ap())
        nc.sync.dma_start(out=alc_sb, in_=alcD.ap())
        nc.sync.dma_start(out=binm, in_=binmD.ap())
        nc.sync.dma_start(out=dw, in_=dwD.ap())

        # ---------------- P1: Q/K projections (column halves) ------------
        # both halves accumulate interleaved; half 0 ([0:1024), all the A
        # sweep needs) drains first so attention exps start ~8us earlier
        with tc.tile_pool(name="qkv_ps", bufs=1, space="PSUM") as pqk:
            psQh = [pqk.tile([128, 1024], F32, tag=f"pq{hf}", bufs=1,
                             name=f"psQ{hf}") for hf in range(2)]
            psKh = [pqk.tile([128, 1024], F32, tag=f"pk{hf}", bufs=1,
                             name=f"psK{hf}") for hf in range(2)]
            # p-state warm-up: keep PE continuously busy from t~0.8us so the
            # real matmuls dispatch at full clock (results are overwritten by
            # the first start=True accumulation below)
            for _ in range(7):
                nc.tensor.matmul(psQh[0][:, 0:512],
                                 lhsT=ones_row[0:1, 0:128],
                                 rhs=ones_row[0:1, 0:512],
                                 start=True, stop=True)
            for c in range(NHC):
                for hf in range(2):
                    for n in range(2):
                        nc.tensor.matmul(
                            psQh[hf][:, n * 512:(n + 1) * 512],
                            lhsT=wQ[:, c * 128:(c + 1) * 128],
                            rhs=xh[c][:, hf * 1024 + n * 512:
                                       hf * 1024 + (n + 1) * 512],
                            start=(c == 0), stop=False)
                for hf in range(2):
                    for n in range(2):
                        nc.tensor.matmul(
                            psKh[hf][:, n * 512:(n + 1) * 512],
                            lhsT=wK[:, c * 128:(c + 1) * 128],
                            rhs=xh[c][:, hf * 1024 + n * 512:
                                       hf * 1024 + (n + 1) * 512],
                            start=(c == 0), stop=False)
            # bias as rank-1 updates so the drains become plain copies
            # that split across DVE and Act in parallel
            for hf in range(2):
                for n in range(2):
                    nc.tensor.matmul(
                        psQh[hf][:, n * 512:(n + 1) * 512],
                        lhsT=bw_sb[0:1, 0:128],
                        rhs=ones_row[:, 0:512],
                        start=False, stop=True)
                    nc.tensor.matmul(
                        psKh[hf][:, n * 512:(n + 1) * 512],
                        lhsT=bw_sb[0:1, 128:256],
                        rhs=ones_row[:, 0:512],
                        start=False, stop=True)
            for hf in range(2):
                lo = hf * 1024
                nc.vector.tensor_copy(out=qaugP[:, lo:lo + 1024],
                                      in_=psQh[hf])
                nc.scalar.activation(out=kaugP[:, lo:lo + 1024],
                                     in_=psKh[hf],
                                     func=mybir.ActivationFunctionType.Copy,
                                     bias=0.0, scale=1.0)

        # ---------------- P2: A-sweep scores/exp + V, interleaved ----------
        # A-sweep probs are saved to dedicated tiles; their ctx matmuls are
        # deferred to P3 where they fill PE slack under the B-sweep.
        probsA = ctx.enter_context(tc.tile_pool(name="probsA", bufs=1))
        pA = {}
        with tc.tile_pool(name="ps", bufs=1, space="PSUM") as pp:
            vps = [pp.tile([128, 8, 128], F32, tag="vp", bufs=2,
                           name=f"vp{b}") for b in range(2)]
            for wi, (h, c) in enumerate([(h, c) for c in range(8)
                                         for h in range(2)]):
                base = 128 * c
                n = 1024 - base
                p0 = 64 * h
                ps = pp.tile([128, 1024], F32, tag="sc", bufs=2, name="psA")
                for s in range(0, n, 512):
                    sl = min(512, n - s)
                    nc.tensor.matmul(
                        ps[:, s:s + sl],
                        lhsT=kaugP[p0:p0 + 64, base:base + 128],
                        rhs=qaugP[p0:p0 + 64, base + s:base + s + sl],
                        start=True, stop=True)
                pb = probsA.tile([128, 1024], BF16, tag=f"pA{wi}",
                                 name=f"pA{wi}")
                col = 16 * h + c
                nc.scalar.activation(out=pb[:, 0:n], in_=ps[:, 0:n],
                                     func=mybir.ActivationFunctionType.Exp,
                                     bias=alc_sb[:, col:col + 1], scale=1.0)
                nc.vector.tensor_mul(out=pb[:, 0:128], in0=pb[:, 0:128],
                                     in1=binm)
                pA[(h, c)] = pb
                # one V token-tile per window (PE filler under Act exps)
                t = wi
                b, ti = t // 8, t % 8
                for cc in range(NHC):
                    nc.tensor.matmul(
                        vps[b][:, ti, :],
                        lhsT=xh[cc][:, t * 128:(t + 1) * 128],
                        rhs=wV[:, cc * 128:(cc + 1) * 128],
                        start=(cc == 0), stop=False)
                nc.tensor.matmul(vps[b][:, ti, :], lhsT=ones_row[:, 0:128],
                                 rhs=bv_sb, start=False, stop=True)
                if ti == 7:
                    for h2 in range(2):
                        nc.vector.tensor_copy(
                            out=vaug[h2][:, 8 * b:8 * (b + 1), 0:64],
                            in_=vps[b][:, :, 64 * h2:64 * h2 + 64])

            # ------------- P3: B-sweep + deferred A-ctx + dense -------------

            def _dense_tile(t, use_act=False):
                psd = pp.tile([128, 1024], F32, tag="sc", bufs=2, name="psd")
                for n in range(2):
                    nc.tensor.matmul(
                        psd[:, n * 512:(n + 1) * 512],
                        lhsT=ctxT[:, t * 128:(t + 1) * 128],
                        rhs=dw[:, n * 512:(n + 1) * 512],
                        start=True, stop=True)
                stg = outp.tile([128, 1024], BF16, tag="og")
                if use_act:
                    nc.scalar.activation(
                        out=stg, in_=psd,
                        func=mybir.ActivationFunctionType.Copy,
                        bias=0.0, scale=1.0)
                else:
                    nc.vector.tensor_copy(out=stg, in_=psd)
                nc.sync.dma_start(out=outD.ap()[:, t * H:(t + 1) * H],
                                  in_=stg)

            def _scores(h, c, qa, qb):
                base = 128 * c
                n = qb - qa
                p0 = 64 * h
                ps = pp.tile([128, 1024], F32, tag="sc", bufs=2, name="ps")
                for s in range(0, n, 512):
                    sl = min(512, n - s)
                    nc.tensor.matmul(
                        ps[:, s:s + sl],
                        lhsT=kaugP[p0:p0 + 64, base:base + 128],
                        rhs=qaugP[p0:p0 + 64, qa + s:qa + s + sl],
                        start=True, stop=True)
                pb = probs.tile([128, 1024], BF16, tag="pb")
                col = 16 * h + c
                nc.scalar.activation(out=pb[:, 0:n], in_=ps[:, 0:n],
                                     func=mybir.ActivationFunctionType.Exp,
                                     bias=alc_sb[:, col:col + 1], scale=1.0)
                if qa == base:
                    nc.vector.tensor_mul(out=pb[:, 0:128], in0=pb[:, 0:128],
                                         in1=binm)
                return pb

            def _ctx(h, pc, c, qa, qb, pb):
                n = qb - qa
                slices = []
                if qa == 128 * c:
                    slices.append((0, 128, True))
                    p = 128
                else:
                    p = 0
                while p < n:
                    nxt = min(n, ((qa + p) // 512 + 1) * 512 - qa)
                    slices.append((p, nxt - p, False))
                    p = nxt
                off = qa if qa < 1024 else qa - 1024
                for (s, sl, stp) in slices:
                    o = off + s
                    nc.tensor.matmul(pc[0:65, o // 128:(o + sl) // 128, :],
                                     lhsT=vaug[h][:, c, 0:65],
                                     rhs=pb[:, s:s + sl],
                                     start=(c == 0), stop=stp,
                                     skip_group_check=True)

            def _normalize(pc, src0, h, dst0, n):
                a, b2 = src0 // 128, (src0 + n) // 128
                rec = work.tile([1, 1024], F32, tag="rec")
                nc.vector.reciprocal(out=rec[:, 0:n], in_=pc[64:65, a:b2, :])
                recb = work.tile([64, 1024], F32, tag="recb")
                nc.gpsimd.partition_broadcast(recb[:, 0:n], rec[0:1, 0:n],
                                              channels=64)
                nc.vector.tensor_mul(
                    out=ctxT[64 * h:64 * h + 64, dst0:dst0 + n],
                    in0=pc[0:64, a:b2, :], in1=recb[:, 0:n])

            pcA = [pp.tile([128, 8, 128], F32, tag="vp", bufs=2,
                           name=f"pcA{h}") for h in range(2)]
            pcB = [pp.tile([128, 8, 128], F32, tag="vp", bufs=2,
                           name=f"pcB{h}") for h in range(2)]
            winsB = [(h, c, (1024 if c < 8 else 128 * c), 2048)
                     for c in range(16) for h in range(2)]
            pendq = []
            actx = [(h, c) for c in range(8) for h in range(2)]
            nd = 0
            avail = 0
            na = 0
            for wi, (h, c, qa, qb) in enumerate(winsB):
                pb = _scores(h, c, qa, qb)
                pendq.append((h, pcB[h], c, qa, qb, pb))
                # A-ctx fillers: two per window over the first 8 windows
                if wi < 8:
                    for _ in range(2):
                        ah, ac = actx[na]
                        _ctx(ah, pcA[ah], ac, 128 * ac, 1024, pA[(ah, ac)])
                        na += 1
                # B-ctx: deferred until pcA is consumed (its psum banks are
                # reused by pcB); catch up 3 per window
                if wi >= 12:
                    for _ in range(4):
                        if len(pendq) > 1:
                            _ctx(*pendq.pop(0))
                if wi == 8:
                    _normalize(pcA[0], 0, 0, 0, 1024)
                    _normalize(pcA[1], 0, 1, 0, 1024)
                    avail = 8
                if wi == 24:
                    _normalize(pcB[0], 0, 0, 1024, 512)
                    _normalize(pcB[1], 0, 1, 1024, 512)
                    avail = 12
                if wi in (26, 28, 30):
                    k = (wi - 2) // 2 - 8   # 4, 5, 6
                    _normalize(pcB[0], 128 * k, 0, 1024 + 128 * k, 128)
                    _normalize(pcB[1], 128 * k, 1, 1024 + 128 * k, 128)
                    avail = 9 + k
                if wi >= 10 and nd < avail:
                    _dense_tile(nd, use_act=(nd >= 6))
                    nd += 1
            while pendq:
                _ctx(*pendq.pop(0))
            if nd < 15:
                _dense_tile(nd, use_act=True)
                nd += 1
            _normalize(pcB[0], 896, 0, 1920, 128)
            _normalize(pcB[1], 896, 1, 1920, 128)
            while nd < 16:
                _dense_tile(nd, use_act=(nd % 2 == 1))
                nd += 1
    nc.compile()
    return nc


# ----------------------------------------------------------------------------
# L3: LN2 + MLP, 4 token groups x 2-way tensor-parallel over 4H
# ----------------------------------------------------------------------------

def build_l3():
    nc = bacc.Bacc("TRN2", target_bir_lowering=False, debug=False,
                   num_devices=NCORE)
    # xres: [p, 1024t + f] = attn_out token 128t+p (512 tokens per core)
    xresD = nc.dram_tensor("xres", [128, 4 * H], BF16, kind="ExternalInput")
    # f1T: m-major packing: [p, 1024*m + 128*c + j] = fc1_eff.T[128c+p, 2048*half + 128m + j]
    f1TD = nc.dram_tensor("f1T", [4, 128, 4 * H], BF16, kind="ExternalInput")
    b1D = nc.dram_tensor("b1c", [128, 16], F32, kind="ExternalInput")
    # f2T: chunk m at cols [1024m:1024(m+1)): fc2_w.T[2048*half+128m+p, f]
    f2TD = nc.dram_tensor("f2T", [4, 128, 4 * H], BF16, kind="ExternalInput")
    outP = nc.dram_tensor("outP", [128, 4 * H], BF16, kind="ExternalOutput")

    with tile.TileContext(nc) as tc, ExitStack() as ctx:
        singles = ctx.enter_context(tc.tile_pool(name="singles", bufs=1))
        stat = ctx.enter_context(tc.tile_pool(name="stat", bufs=2))
        work = ctx.enter_context(tc.tile_pool(name="work", bufs=2))
        hp = ctx.enter_context(tc.tile_pool(name="hp", bufs=1))
        outp = ctx.enter_context(tc.tile_pool(name="outp", bufs=2))

        ident = singles.tile([128, 128], BF16)
        make_identity(nc, ident)
        epst = singles.tile([128, 1], F32)
        nc.vector.memset(epst, EPS)
        warm = singles.tile([1, 1], F32)
        nc.scalar.activation(out=warm, in_=epst[0:1, 0:1],
                             func=mybir.ActivationFunctionType.Sqrt,
                             bias=0.0, scale=1.0)

        xres = singles.tile([128, 4, H], BF16)
        b1c = singles.tile([128, 16], F32)
        f1 = [singles.tile([128, 4 * H], BF16, tag=f"f1{g}", name=f"f1{g}")
              for g in range(4)]
        f2 = [singles.tile([128, 4 * H], BF16, tag=f"f2{g}", name=f"f2{g}")
              for g in range(4)]
        # interleave: first two xres tiles (gate LN2 for fc1's first half),
        # then the first fc1 group, then the rest
        nc.sync.dma_start(out=xres[:, 0, :], in_=xresD.ap()[:, 0:H])
        nc.sync.dma_start(out=xres[:, 1, :], in_=xresD.ap()[:, H:2 * H])
        nc.sync.dma_start(out=b1c, in_=b1D.ap())
        nc.sync.dma_start(out=f1[0], in_=f1TD.ap()[0])
        nc.sync.dma_start(out=xres[:, 2, :], in_=xresD.ap()[:, 2 * H:3 * H])
        nc.sync.dma_start(out=xres[:, 3, :], in_=xresD.ap()[:, 3 * H:4 * H])
        for g in range(1, 4):
            nc.sync.dma_start(out=f1[g], in_=f1TD.ap()[g])
        for g in range(4):
            nc.sync.dma_start(out=f2[g], in_=f2TD.ap()[g])

        # xh2T: [p, c, 128t+j] = xhat^T chunk c
        xh2T = singles.tile([128, NHC, 512], BF16)

        ones_pe = singles.tile([1, 512], BF16)
        nc.vector.memset(ones_pe, 1.0)
        with tc.tile_pool(name="ps", bufs=1, space="PSUM") as pp:
            pwarm = pp.tile([128, 512], F32, tag="f1", bufs=3, name="pwarm")
            for _ in range(9):
                nc.tensor.matmul(pwarm, lhsT=ones_pe[0:1, 0:128],
                                 rhs=ones_pe[0:1, 0:512],
                                 start=True, stop=True)
            for t in range(4):
                st = stat.tile([128, 2, 6], F32, tag="bnst")
                nc.vector.bn_stats(out=st[:, 0, :], in_=xres[:, t, 0:512])
                nc.vector.bn_stats(out=st[:, 1, :], in_=xres[:, t, 512:1024])
                mv = stat.tile([128, 2], F32, tag="bnmv")
                nc.vector.bn_aggr(out=mv, in_=st)
                rstd = stat.tile([128, 1], F32, tag="rstd")
                nc.scalar.activation(out=rstd, in_=mv[:, 1:2],
                                     func=mybir.ActivationFunctionType.Sqrt,
                                     bias=epst, scale=1.0)
                nc.vector.reciprocal(out=rstd, in_=rstd)
                xh = work.tile([128, H], BF16, tag="xhat")
                nc.vector.tensor_scalar(out=xh, in0=xres[:, t, :],
                                        scalar1=mv[:, 0:1], scalar2=rstd,
                                        op0=mybir.AluOpType.subtract,
                                        op1=mybir.AluOpType.mult)
                tp = pp.tile([128, 1024], BF16, tag="tp", bufs=1)
                for c in range(NHC):
                    nc.tensor.transpose(tp[:, c * 128:(c + 1) * 128],
                                        xh[:, c * 128:(c + 1) * 128], ident)
                nc.vector.tensor_copy(out=xh2T[:, :, t * 128:(t + 1) * 128],
                                      in_=tp)

            hts = {}

            def _fc1(m):
                ps = pp.tile([128, 512], F32, tag="f1", bufs=3, name="psf1")
                # token-quarters: the first fc1 only waits on LN2 of tile 0
                for tg in range(4):
                    for c in range(NHC):
                        nc.tensor.matmul(
                            ps[:, tg * 128:(tg + 1) * 128],
                            lhsT=f1[m // 4][:, (m % 4) * 1024 + c * 128:
                                            (m % 4) * 1024 + (c + 1) * 128],
                            rhs=xh2T[:, c, tg * 128:(tg + 1) * 128],
                            start=(c == 0), stop=(c == NHC - 1))
                ht = hp.tile([128, 512], BF16, tag=f"h{m}")
                if os.environ.get("BLOOM_SIM"):
                    u = work.tile([128, 512], F32, tag="gelu_u")
                    nc.vector.tensor_scalar_add(out=u, in0=ps,
                                                scalar1=b1c[:, m:m + 1])
                    s2 = work.tile([128, 512], F32, tag="gelu_s")
                    nc.vector.tensor_mul(out=s2, in0=u, in1=u)
                    nc.vector.tensor_scalar(out=s2, in0=s2,
                                            scalar1=0.035677408145115,
                                            scalar2=0.7978845608028654,
                                            op0=mybir.AluOpType.mult,
                                            op1=mybir.AluOpType.add)
                    nc.vector.tensor_mul(out=s2, in0=s2, in1=u)
                    nc.scalar.activation(out=s2, in_=s2,
                                         func=mybir.ActivationFunctionType.Tanh,
                                         bias=0.0, scale=1.0)
                    nc.vector.tensor_scalar(out=s2, in0=s2, scalar1=1.0,
                                            scalar2=0.5,
                                            op0=mybir.AluOpType.add,
                                            op1=mybir.AluOpType.mult)
                    nc.vector.tensor_mul(out=ht, in0=s2, in1=u)
                else:
                    nc.scalar.activation(
                        out=ht, in_=ps,
                        func=mybir.ActivationFunctionType.Gelu_apprx_tanh,
                        bias=b1c[:, m:m + 1], scale=1.0)
                hts[m] = ht

            def _fc2(psf2, m, tpair):
                ht = hts[m]
                for ti, t in enumerate(tpair):
                    for n in range(2):
                        nc.tensor.matmul(
                            psf2[ti][:, n * 512:(n + 1) * 512],
                            lhsT=ht[:, t * 128:(t + 1) * 128],
                            rhs=f2[m // 4][:, (m % 4) * 1024 + n * 512:
                                           (m % 4) * 1024 + (n + 1) * 512],
                            start=(m == 0), stop=(m == 15))

            def _drain(psf2, tpair):
                for ti, t in enumerate(tpair):
                    stg = outp.tile([128, 1024], BF16, tag="og")
                    if t % 2 == 0:
                        nc.vector.tensor_copy(out=stg, in_=psf2[ti])
                    else:
                        nc.scalar.activation(
                            out=stg, in_=psf2[ti],
                            func=mybir.ActivationFunctionType.Copy,
                            bias=0.0, scale=1.0)
                    nc.sync.dma_start(out=outP.ap()[:, t * H:(t + 1) * H],
                                      in_=stg)

            # pass 1: fc1 all m, fc2 into token tiles 0,1 (staggered)
            psf2 = [pp.tile([128, 1024], F32, tag=f"f2_{t}", bufs=1,
                            name=f"psf2_{t}")
                    for t in range(2)]
            _fc1(0)
            for m in range(16):
                if m + 1 < 16:
                    _fc1(m + 1)
                _fc2(psf2, m, (0, 1))
            _drain(psf2, (0, 1))
            # passes 2/3: fc2 for token tiles 2 then 3 (staggered drains)
            psf2b = [pp.tile([128, 1024], F32, tag="f2_0", bufs=1,
                             name="psf2b")]
            for m in range(16):
                _fc2(psf2b, m, (2,))
            _drain(psf2b, (2,))
            psf2c = [pp.tile([128, 1024], F32, tag="f2_1", bufs=1,
                             name="psf2c")]
            for m in range(16):
                _fc2(psf2c, m, (3,))
            _drain(psf2c, (3,))
    nc.compile()
    return nc


# ----------------------------------------------------------------------------
# host orchestration
# ----------------------------------------------------------------------------

_NC_CACHE = {}
_BUILDERS = {"l1": build_l1, "l2": build_l2, "l3": build_l3}


def _get_nc(name):
    if name not in _NC_CACHE:
        _NC_CACHE[name] = _BUILDERS[name]()
    return _NC_CACHE[name]


def _run(nc, in_maps):
    if os.environ.get("BLOOM_SIM"):
        from concourse.bass_interp import CoreSim
        results = []
        for m in in_maps:
            sim = CoreSim(nc, trace=False)
            for k, v in m.items():
                sim.tensor(k)[:] = v
            sim.simulate(check_with_hw=False)
            outs = {}
            for alloc in nc.m.functions[0].allocations:
                if getattr(alloc, "kind", None) == "ExternalOutput":
                    nm = alloc.memorylocations[0].name
                    outs[nm] = np.array(sim.tensor(nm))
            results.append(outs)
        return results
    from concourse.bass_utils import run_bass_kernel_spmd
    res = run_bass_kernel_spmd(nc, in_maps, core_ids=list(range(NCORE)))
    return res.results


def _prep_weights(ln1_g, ln1_b, qkv_w, qkv_b, dense_w, dense_b,
                  ln2_g, ln2_b, fc1_w, fc1_b, fc2_w, fc2_b):
    qkv_w = np.asarray(qkv_w, np.float32)
    qkv_b = np.asarray(qkv_b, np.float32)
    w_eff = qkv_w * np.asarray(ln1_g, np.float32)[None, :]
    b_eff = qkv_b + qkv_w @ np.asarray(ln1_b, np.float32)
    w3 = w_eff.reshape(NH, 3 * HD, H)
    b3 = b_eff.reshape(NH, 3 * HD)
    wq = w3[:, :HD, :] / NORM
    wk = w3[:, HD:2 * HD, :]
    wv = w3[:, 2 * HD:, :]
    bq = b3[:, :HD] / NORM
    bk = b3[:, HD:2 * HD]
    bv = b3[:, 2 * HD:]

    def pack_lhsT(w_pair):
        """[128 f, 1024 h] -> [p, 128c + f] = w_pair[f, 128c + p]"""
        return np.ascontiguousarray(
            w_pair.T.reshape(NHC, 128, 128).transpose(1, 0, 2)
            .reshape(128, NHC * 128)).astype(NBF)

    wQ_i, wK_i, wV_i, bcol_i, bvr_i = [], [], [], [], []
    for i in range(NCORE):
        h0, h1 = 2 * i, 2 * i + 1
        wQ_i.append(pack_lhsT(np.concatenate([wq[h0], wq[h1]], 0)))
        wK_i.append(pack_lhsT(np.concatenate([wk[h0], wk[h1]], 0)))
        wV_i.append(pack_lhsT(np.concatenate([wv[h0], wv[h1]], 0)))
        bcol_i.append(np.concatenate(
            [bq[h0], bq[h1], bk[h0], bk[h1]]).reshape(1, 256).astype(NBF))
        bvr_i.append(np.concatenate([bv[h0], bv[h1]])
                     .reshape(1, 128).astype(NBF))

    dwT = np.ascontiguousarray(np.asarray(dense_w, np.float32).T).astype(NBF)
    db_r = np.asarray(dense_b, np.float32).reshape(1, H)

    f1_eff = np.asarray(fc1_w, np.float32) * np.asarray(ln2_g, np.float32)[None, :]
    b1_eff = np.asarray(fc1_b, np.float32) + np.asarray(fc1_w, np.float32) @ np.asarray(ln2_b, np.float32)
    fc1T = np.ascontiguousarray(f1_eff.T)                           # [H, 4H]
    fc2T = np.ascontiguousarray(np.asarray(fc2_w, np.float32).T)    # [4H, H]
    b2_r = np.asarray(fc2_b, np.float32).reshape(1, H)

    # L3 packings, per half
    f1T_half, b1_half, f2T_half = [], [], []
    for half in range(2):
        cols = slice(half * 2 * H, (half + 1) * 2 * H)
        f1h = fc1T[:, cols]                                         # [1024, 2048]
        # f1TD[g, p, 1024*(m%4) + 128c + j] = f1h[128c + p, 128m + j]
        a = (f1h.reshape(NHC, 128, 16, 128)      # [c, p, m, j]
             .transpose(2, 1, 0, 3)              # [m, p, c, j]
             .reshape(4, 4, 128, NHC * 128)      # [g, m%4, p, c*j]
             .transpose(0, 2, 1, 3)              # [g, p, m%4, c*j]
             .reshape(4, 128, 4 * H))
        f1T_half.append(np.ascontiguousarray(a.astype(NBF)))
        b1h = b1_eff[half * 2 * H:(half + 1) * 2 * H]
        b1_half.append(np.ascontiguousarray(
            b1h.reshape(16, 128).T).astype(np.float32))
        f2h = fc2T[half * 2 * H:(half + 1) * 2 * H, :]              # [2048, 1024]
        # f2TD[g, p, 1024*(m%4) + f] = f2h[128m + p, f]
        b = (f2h.reshape(4, 4, 128, H)           # [g, m%4, p, f]
             .transpose(0, 2, 1, 3)              # [g, p, m%4, f]
             .reshape(4, 128, 4 * H))
        f2T_half.append(np.ascontiguousarray(b.astype(NBF)))
    return dict(wQ=wQ_i, wK=wK_i, wV=wV_i, bcol=bcol_i, bvr=bvr_i, db=db_r,
                dwT=dwT, f1T_half=f1T_half, b1_half=b1_half,
                f2T_half=f2T_half, b2=b2_r)


def _tri_mask():
    k = np.arange(QB)[:, None]
    q = np.arange(QB)[None, :]
    return np.where(k <= q, 1.0, 0.0).astype(NBF)   # [k, q] allowed k<=q


def kernel(hidden_states, attention_mask, alibi,
           ln1_g, ln1_b, qkv_w, qkv_b, dense_w, dense_b,
           ln2_g, ln2_b, fc1_w, fc1_b, fc2_w, fc2_b):
    X = np.asarray(hidden_states, np.float32).reshape(S, H)
    alibi_np = np.asarray(alibi, np.float32).reshape(NH, S)
    W = _prep_weights(ln1_g, ln1_b, qkv_w, qkv_b, dense_w, dense_b,
                      ln2_g, ln2_b, fc1_w, fc1_b, fc2_w, fc2_b)

    # ---------------- L1: LN1 + transpose ----------------
    nc1 = _get_nc("l1")
    in1 = []
    for i in range(NCORE):
        a, b = _blocks(i)
        xi = np.concatenate([X[a * QB:(a + 1) * QB], X[b * QB:(b + 1) * QB]], 0)
        xp = np.ascontiguousarray(
            xi.reshape(2, 128, H).transpose(1, 0, 2)).astype(NBF)
        in1.append(dict(x=xp))
    r1 = _run(nc1, in1)

    # ---------------- host gather: xhat^T chunk-major ----------------
    xhG = np.zeros((NHC, 128, S), NBF)
    for i in range(NCORE):
        a, b = _blocks(i)
        r = r1[i]["xhT"].reshape(2, 128, NHC, 128)
        xhG[:, :, a * QB:(a + 1) * QB] = r[0].transpose(1, 0, 2)
        xhG[:, :, b * QB:(b + 1) * QB] = r[1].transpose(1, 0, 2)
    xhG = np.ascontiguousarray(xhG)

    # ---------------- L2: QKV + attention + dense partial ----------------
    binm = _tri_mask()
    dwT = W["dwT"]
    nc2 = _get_nc("l2")
    in2 = []
    for i in range(NCORE):
        alc = np.zeros((128, 32), np.float32)
        for j in range(2):
            alc[:, 16 * j:16 * (j + 1)] = \
                alibi_np[2 * i + j].reshape(16, 128).T
        dwi = np.ascontiguousarray(dwT[i * 128:(i + 1) * 128, :]).astype(NBF)
        in2.append(dict(xh=xhG, wQ=W["wQ"][i], wK=W["wK"][i], wV=W["wV"][i],
                        bw=W["bcol"][i], bvr=W["bvr"][i], alc=alc,
                        binm=binm, dw=dwi))
    r2 = _run(nc2, in2)

    # host reduce: attn_out = sum of dense partials + residual + dense bias
    attn_out = X + W["db"]
    for i in range(NCORE):
        attn_out = attn_out + r2[i]["outD"].astype(np.float32) \
            .reshape(128, 16, H).transpose(1, 0, 2).reshape(S, H)

    # ---------------- L3 ----------------
    nc3 = _get_nc("l3")
    in3 = []
    attn_bf = attn_out.astype(NBF)
    for i in range(NCORE):
        g, half = i // 2, i % 2
        xg = attn_bf[512 * g:512 * (g + 1)]        # [512, H]
        xres = np.ascontiguousarray(
            xg.reshape(4, 128, H).transpose(1, 0, 2).reshape(128, 4 * H))
        in3.append(dict(xres=xres, f1T=W["f1T_half"][half],
                        b1c=W["b1_half"][half], f2T=W["f2T_half"][half]))
    r3 = _run(nc3, in3)

    out = np.empty((S, H), np.float32)
    for g in range(4):
        p = r3[2 * g]["outP"].astype(np.float32) + \
            r3[2 * g + 1]["outP"].astype(np.float32)
        out[512 * g:512 * (g + 1)] = \
            p.reshape(128, 4, H).transpose(1, 0, 2).reshape(512, H) \
            + attn_out[512 * g:512 * (g + 1)] + W["b2"]
    return out.reshape(1, S, H)


# revision 64
# speedup vs baseline: 1.0009x; 1.0004x over previous
"""BloomBlock on 8 TRN2 NeuronCores — 3-launch structure.

  * L1 (data-parallel over tokens): LN1 (folded into weights on host) +
    QKV projection for each core's 256 tokens (blocks i and 15-i).
  * Host: all-gather Q/K/V, regroup per head.
  * L2 (tensor-parallel over heads): each core owns 2 heads for ALL 2048
    queries. Exact-causal attention (no padded key slots): per key chunk
    c, only queries >= 128c are scored. Transposed-score layout (keys on
    partitions, queries on free dim; softmax denominator via an appended
    ones-column on V; alibi via a bias row on K matched with a ones row
    on Q). Diagonal chunks get a post-exp binary stair mask. Fused
    row-parallel dense: each core emits a partial dense output over all
    tokens from its 2 heads' context.
  * Host: reduce dense partials + residual + dense bias -> attn_out.
  * L3 (4 token groups x 2-way tensor-parallel MLP): each core runs LN2
    on its group's 512 tokens and computes fc1/gelu/fc2 for half the 4H
    features; partial fc2 outputs are reduced on host with residual2.
"""

import os
from contextlib import ExitStack

import ml_dtypes
import numpy as np

import concourse.bass as bass
import concourse.tile as tile
from concourse import bacc, mybir
from concourse.masks import make_identity

BF16 = mybir.dt.bfloat16
F32 = mybir.dt.float32
NBF = ml_dtypes.bfloat16

S, H, NH, HD = 2048, 1024, 16, 64
NCORE = 8
QB = 128          # token/key chunk size
SC = 2 * QB       # tokens per core in L1
NSLOT = 16
NHC = H // 128    # hidden chunks
EPS = 1e-5
NORM = float(np.sqrt(HD))  # 8.0 (LAYER_NUMBER = 1)


def _blocks(i):
    return (i, 15 - i)


# ----------------------------------------------------------------------------
# L1: LN1 + QKV, data-parallel over tokens (unchanged from baseline)
# ----------------------------------------------------------------------------

def build_l1():
    """Tiny launch: LN1 + transpose only (token-DP, 256 tokens/core)."""
    nc = bacc.Bacc("TRN2", target_bir_lowering=False, debug=False,
                   num_devices=NCORE)
    # x token-major packed: [p, t, f] = token 128t+p (bf16)
    x = nc.dram_tensor("x", [128, 2, H], BF16, kind="ExternalInput")
    # xhT: [t, p, 128c + j] = xhat^T[128c+p, token 128t+j]
    xhT = nc.dram_tensor("xhT", [2, 128, NHC * 128], BF16,
                         kind="ExternalOutput")

    with tile.TileContext(nc) as tc, ExitStack() as ctx:
        singles = ctx.enter_context(tc.tile_pool(name="singles", bufs=1))
        stat = ctx.enter_context(tc.tile_pool(name="stat", bufs=4))
        work = ctx.enter_context(tc.tile_pool(name="work", bufs=2))

        ident = singles.tile([128, 128], BF16)
        make_identity(nc, ident)
        epst = singles.tile([128, 1], F32)
        nc.vector.memset(epst, EPS)
        warm = singles.tile([1, 1], F32)
        nc.scalar.activation(out=warm, in_=epst[0:1, 0:1],
                             func=mybir.ActivationFunctionType.Sqrt,
                             bias=0.0, scale=1.0)
        xt = singles.tile([128, 2, H], BF16)
        nc.sync.dma_start(out=xt[:, 0, :], in_=x.ap()[:, 0, :])
        nc.sync.dma_start(out=xt[:, 1, :], in_=x.ap()[:, 1, :])
        stage = singles.tile([128, 2, NHC * 128], BF16)

        with tc.tile_pool(name="tp_ps", bufs=2, space="PSUM") as tp_ps:
            # p-state warm-up (overwritten by the real transposes)
            pw = tp_ps.tile([128, 512], F32, tag="pw", bufs=1, name="pw")
            for _ in range(20):
                nc.tensor.matmul(pw[:, 0:128], lhsT=ident[0:1, :],
                                 rhs=ident[0:1, :], start=True, stop=True)
            # both stat chains first so the DVE stream never gates t=1
            rstds = []
            for t in range(2):
                st = stat.tile([128, 2, 6], F32, tag="bnst")
                nc.vector.bn_stats(out=st[:, 0, :], in_=xt[:, t, 0:512])
                nc.vector.bn_stats(out=st[:, 1, :], in_=xt[:, t, 512:1024])
                mv = stat.tile([128, 2], F32, tag="bnmv")
                nc.vector.bn_aggr(out=mv, in_=st)
                rstd = stat.tile([128, 1], F32, tag="rstd")
                nc.scalar.activation(out=rstd, in_=mv[:, 1:2],
                                     func=mybir.ActivationFunctionType.Sqrt,
                                     bias=epst, scale=1.0)
                nc.vector.reciprocal(out=rstd, in_=rstd)
                rstds.append((mv, rstd))
            for t in range(2):
                mv, rstd = rstds[t]
                xh = work.tile([128, H], BF16, tag="xhat")
                nc.vector.tensor_scalar(out=xh, in0=xt[:, t, :],
                                        scalar1=mv[:, 0:1], scalar2=rstd,
                                        op0=mybir.AluOpType.subtract,
                                        op1=mybir.AluOpType.mult)
                tp = tp_ps.tile([128, NHC * 128], BF16, tag="tp")
                for c in range(NHC):
                    nc.tensor.transpose(tp[:, c * 128:(c + 1) * 128],
                                        xh[:, c * 128:(c + 1) * 128], ident)
                nc.vector.tensor_copy(out=stage[:, t, :], in_=tp)
                nc.sync.dma_start(out=xhT.ap()[t], in_=stage[:, t, :])
    nc.compile()
    return nc


# ----------------------------------------------------------------------------
# L2: exact-causal attention, tensor-parallel over heads (2 heads/core),
#     fused row-parallel dense partial.
# ----------------------------------------------------------------------------

def build_l2():
    """QKV (2 heads/core over all tokens) + exact-causal attention +
    row-parallel dense partial."""
    nc = bacc.Bacc("TRN2", target_bir_lowering=False, debug=False,
                   num_devices=NCORE)
    # xhat^T chunk-major (replicated input): [c, p, tok]
    xhD = nc.dram_tensor("xh", [NHC, 128, S], BF16, kind="ExternalInput")
    # lhsT weight packs, chunk c at cols [128c,128c+128): [q0|q1], [k0|k1]
    wQD = nc.dram_tensor("wQ", [128, NHC * 128], BF16, kind="ExternalInput")
    wKD = nc.dram_tensor("wK", [128, NHC * 128], BF16, kind="ExternalInput")
    # v rhs pack, chunk c: [128 h, [v0|v1] 128]
    wVD = nc.dram_tensor("wV", [128, NHC * 128], BF16, kind="ExternalInput")
    bwD = nc.dram_tensor("bw", [1, 256], BF16, kind="ExternalInput")
    bvrD = nc.dram_tensor("bvr", [1, 128], BF16, kind="ExternalInput")
    # alibi columns: [p, 16h + c] = alibi[head h, key 128c+p]
    alcD = nc.dram_tensor("alc", [128, 32], F32, kind="ExternalInput")
    binmD = nc.dram_tensor("binm", [QB, QB], BF16, kind="ExternalInput")
    dwD = nc.dram_tensor("dw", [128, H], BF16, kind="ExternalInput")
    # dense partial, token-major packed: [p, 1024*t + f] = token 128t+p
    outD = nc.dram_tensor("outD", [128, 16 * H], BF16, kind="ExternalOutput")

    with tile.TileContext(nc) as tc, ExitStack() as ctx:
        singles = ctx.enter_context(tc.tile_pool(name="singles", bufs=1))
        probs = ctx.enter_context(tc.tile_pool(name="probs", bufs=16))
        work = ctx.enter_context(tc.tile_pool(name="work", bufs=3))
        outp = ctx.enter_context(tc.tile_pool(name="outp", bufs=4))

        binm = singles.tile([QB, QB], BF16)
        dw = singles.tile([128, H], BF16)
        ctxT = singles.tile([128, S], BF16)
        qaugP = singles.tile([128, S], BF16)
        kaugP = singles.tile([128, S], BF16)
        alc_sb = singles.tile([128, 32], F32)
        bw_sb = singles.tile([1, 256], BF16)
        bv_sb = singles.tile([1, 128], BF16)
        ones_row = singles.tile([1, 1024], BF16)
        nc.vector.memset(ones_row, 1.0)
        vaug = []
        for h in range(2):
            va = singles.tile([128, 16, 65], BF16, tag=f"vaug{h}",
                              name=f"vaug{h}")
            nc.vector.memset(va, 1.0)
            vaug.append(va)
        epst = singles.tile([1, 1], F32)
        nc.vector.memset(epst, EPS)
        warm = singles.tile([1, 1], F32)
        nc.scalar.activation(out=warm, in_=epst,
                             func=mybir.ActivationFunctionType.Exp,
                             bias=0.0, scale=1.0)

        wQ = singles.tile([128, NHC * 128], BF16)
        nc.sync.dma_start(out=wQ, in_=wQD.ap())
        wK = singles.tile([128, NHC * 128], BF16)
        nc.sync.dma_start(out=wK, in_=wKD.ap())
        xh = []
        for c in range(NHC):
            xc = singles.tile([128, S], BF16, tag=f"xh{c}", name=f"xh{c}")
            nc.sync.dma_start(out=xc, in_=xhD.ap()[c])
            xh.append(xc)
        wV = singles.tile([128, NHC * 128], BF16)
        nc.sync.dma_start(out=wV, in_=wVD.ap())
        nc.sync.dma_start(out=bw_sb, in_=bwD.ap())
        nc.sync.dma_start(out=bv_sb, in_=bvrD.ap())
        nc.sync.dma_start(out=alc_sb, in_=alcD.ap())
        nc.sync.dma_start(out=binm, in_=binmD.ap())
        nc.sync.dma_start(out=dw, in_=dwD.ap())

        # ---------------- P1: Q/K projections (column halves) ------------
        # both halves accumulate interleaved; half 0 ([0:1024), all the A
        # sweep needs) drains first so attention exps start ~8us earlier
        with tc.tile_pool(name="qkv_ps", bufs=1, space="PSUM") as pqk:
            psQh = [pqk.tile([128, 1024], F32, tag=f"pq{hf}", bufs=1,
                             name=f"psQ{hf}") for hf in range(2)]
            psKh = [pqk.tile([128, 1024], F32, tag=f"pk{hf}", bufs=1,
                             name=f"psK{hf}") for hf in range(2)]
            # p-state warm-up: keep PE continuously busy from t~0.8us so the
            # real matmuls dispatch at full clock (results are overwritten by
            # the first start=True accumulation below)
            for _ in range(7):
                nc.tensor.matmul(psQh[0][:, 0:512],
                                 lhsT=ones_row[0:1, 0:128],
                                 rhs=ones_row[0:1, 0:512],
                                 start=True, stop=True)
            for c in range(NHC):
                for hf in range(2):
                    for n in range(2):
                        nc.tensor.matmul(
                            psQh[hf][:, n * 512:(n + 1) * 512],
                            lhsT=wQ[:, c * 128:(c + 1) * 128],
                            rhs=xh[c][:, hf * 1024 + n * 512:
                                       hf * 1024 + (n + 1) * 512],
                            start=(c == 0), stop=False)
                for hf in range(2):
                    for n in range(2):
                        nc.tensor.matmul(
                            psKh[hf][:, n * 512:(n + 1) * 512],
                            lhsT=wK[:, c * 128:(c + 1) * 128],
                            rhs=xh[c][:, hf * 1024 + n * 512:
                                       hf * 1024 + (n + 1) * 512],
                            start=(c == 0), stop=False)
            # bias as rank-1 updates so the drains become plain copies
            # that split across DVE and Act in parallel
            for hf in range(2):
                for n in range(2):
                    nc.tensor.matmul(
                        psQh[hf][:, n * 512:(n + 1) * 512],
                        lhsT=bw_sb[0:1, 0:128],
                        rhs=ones_row[:, 0:512],
                        start=False, stop=True)
                    nc.tensor.matmul(
                        psKh[hf][:, n * 512:(n + 1) * 512],
                        lhsT=bw_sb[0:1, 128:256],
                        rhs=ones_row[:, 0:512],
                        start=False, stop=True)
            for hf in range(2):
                lo = hf * 1024
                nc.vector.tensor_copy(out=qaugP[:, lo:lo + 1024],
                                      in_=psQh[hf])
                nc.scalar.activation(out=kaugP[:, lo:lo + 1024],
                                     in_=psKh[hf],
                                     func=mybir.ActivationFunctionType.Copy,
                                     bias=0.0, scale=1.0)

        # ---------------- P2: A-sweep scores/exp + V, interleaved ----------
        # A-sweep probs are saved to dedicated tiles; their ctx matmuls are
        # deferred to P3 where they fill PE slack under the B-sweep.
        probsA = ctx.enter_context(tc.tile_pool(name="probsA", bufs=1))
        pA = {}
        with tc.tile_pool(name="ps", bufs=1, space="PSUM") as pp:
            vps = [pp.tile([128, 8, 128], F32, tag="vp", bufs=2,
                           name=f"vp{b}") for b in range(2)]
            for wi, (h, c) in enumerate([(h, c) for c in range(8)
                                         for h in range(2)]):
                base = 128 * c
                n = 1024 - base
                p0 = 64 * h
                ps = pp.tile([128, 1024], F32, tag="sc", bufs=2, name="psA")
                for s in range(0, n, 512):
                    sl = min(512, n - s)
                    nc.tensor.matmul(
                        ps[:, s:s + sl],
                        lhsT=kaugP[p0:p0 + 64, base:base + 128],
                        rhs=qaugP[p0:p0 + 64, base + s:base + s + sl],
                        start=True, stop=True)
                pb = probsA.tile([128, 1024], BF16, tag=f"pA{wi}",
                                 name=f"pA{wi}")
                col = 16 * h + c
                nc.scalar.activation(out=pb[:, 0:n], in_=ps[:, 0:n],
                                     func=mybir.ActivationFunctionType.Exp,
                                     bias=alc_sb[:, col:col + 1], scale=1.0)
                nc.vector.tensor_mul(out=pb[:, 0:128], in0=pb[:, 0:128],
                                     in1=binm)
                pA[(h, c)] = pb
                # one V token-tile per window (PE filler under Act exps)
                t = wi
                b, ti = t // 8, t % 8
                for cc in range(NHC):
                    nc.tensor.matmul(
                        vps[b][:, ti, :],
                        lhsT=xh[cc][:, t * 128:(t + 1) * 128],
                        rhs=wV[:, cc * 128:(cc + 1) * 128],
                        start=(cc == 0), stop=False)
                nc.tensor.matmul(vps[b][:, ti, :], lhsT=ones_row[:, 0:128],
                                 rhs=bv_sb, start=False, stop=True)
                if ti == 7:
                    for h2 in range(2):
                        nc.vector.tensor_copy(
                            out=vaug[h2][:, 8 * b:8 * (b + 1), 0:64],
                            in_=vps[b][:, :, 64 * h2:64 * h2 + 64])

            # ------------- P3: B-sweep + deferred A-ctx + dense -------------

            def _dense_tile(t, use_act=False):
                psd = pp.tile([128, 1024], F32, tag="sc", bufs=2, name="psd")
                for n in range(2):
                    nc.tensor.matmul(
                        psd[:, n * 512:(n + 1) * 512],
                        lhsT=ctxT[:, t * 128:(t + 1) * 128],
                        rhs=dw[:, n * 512:(n + 1) * 512],
                        start=True, stop=True)
                stg = outp.tile([128, 1024], BF16, tag="og")
                if use_act:
                    nc.scalar.activation(
                        out=stg, in_=psd,
                        func=mybir.ActivationFunctionType.Copy,
                        bias=0.0, scale=1.0)
                else:
                    nc.vector.tensor_copy(out=stg, in_=psd)
                nc.sync.dma_start(out=outD.ap()[:, t * H:(t + 1) * H],
                                  in_=stg)

            def _scores(h, c, qa, qb):
                base = 128 * c
                n = qb - qa
                p0 = 64 * h
                ps = pp.tile([128, 1024], F32, tag="sc", bufs=2, name="ps")
                for s in range(0, n, 512):
                    sl = min(512, n - s)
                    nc.tensor.matmul(
                        ps[:, s:s + sl],
                        lhsT=kaugP[p0:p0 + 64, base:base + 128],
                        rhs=qaugP[p0:p0 + 64, qa + s:qa + s + sl],
                        start=True, stop=True)
                pb = probs.tile([128, 1024], BF16, tag="pb")
                col = 16 * h + c
                nc.scalar.activation(out=pb[:, 0:n], in_=ps[:, 0:n],
                                     func=mybir.ActivationFunctionType.Exp,
                                     bias=alc_sb[:, col:col + 1], scale=1.0)
                if qa == base:
                    nc.vector.tensor_mul(out=pb[:, 0:128], in0=pb[:, 0:128],
                                         in1=binm)
                return pb

            def _ctx(h, pc, c, qa, qb, pb):
                n = qb - qa
                slices = []
                if qa == 128 * c:
                    slices.append((0, 128, True))
                    p = 128
                else:
                    p = 0
                while p < n:
                    nxt = min(n, ((qa + p) // 512 + 1) * 512 - qa)
                    slices.append((p, nxt - p, False))
                    p = nxt
                off = qa if qa < 1024 else qa - 1024
                for (s, sl, stp) in slices:
                    o = off + s
                    nc.tensor.matmul(pc[0:65, o // 128:(o + sl) // 128, :],
                                     lhsT=vaug[h][:, c, 0:65],
                                     rhs=pb[:, s:s + sl],
                                     start=(c == 0), stop=stp,
                                     skip_group_check=True)

            def _normalize(pc, src0, h, dst0, n):
                a, b2 = src0 // 128, (src0 + n) // 128
                rec = work.tile([1, 1024], F32, tag="rec")
                nc.vector.reciprocal(out=rec[:, 0:n], in_=pc[64:65, a:b2, :])
                recb = work.tile([64, 1024], F32, tag="recb")
                nc.gpsimd.partition_broadcast(recb[:, 0:n], rec[0:1, 0:n],
                                              channels=64)
                nc.vector.tensor_mul(
                    out=ctxT[64 * h:64 * h + 64, dst0:dst0 + n],
                    in0=pc[0:64, a:b2, :], in1=recb[:, 0:n])

            pcA = [pp.tile([128, 8, 128], F32, tag="vp", bufs=2,
                           name=f"pcA{h}") for h in range(2)]
            pcB = [pp.tile([128, 8, 128], F32, tag="vp", bufs=2,
                           name=f"pcB{h}") for h in range(2)]
            winsB = [(h, c, (1024 if c < 8 else 128 * c), 2048)
                     for c in range(16) for h in range(2)]
            pendq = []
            actx = [(h, c) for c in range(8) for h in range(2)]
            nd = 0
            avail = 0
            na = 0
            for wi, (h, c, qa, qb) in enumerate(winsB):
                pb = _scores(h, c, qa, qb)
                pendq.append((h, pcB[h], c, qa, qb, pb))
                # A-ctx fillers: two per window over the first 8 windows
                if wi < 8:
                    for _ in range(2):
                        ah, ac = actx[na]
                        _ctx(ah, pcA[ah], ac, 128 * ac, 1024, pA[(ah, ac)])
                        na += 1
                # B-ctx: deferred until pcA is consumed (its psum banks are
                # reused by pcB); catch up 3 per window
                if wi >= 12:
                    for _ in range(4):
                        if len(pendq) > 1:
                            _ctx(*pendq.pop(0))
                if wi == 8:
                    _normalize(pcA[0], 0, 0, 0, 1024)
                    _normalize(pcA[1], 0, 1, 0, 1024)
                    avail = 8
                if wi == 24:
                    _normalize(pcB[0], 0, 0, 1024, 512)
                    _normalize(pcB[1], 0, 1, 1024, 512)
                    avail = 12
                if wi in (26, 28, 30):
                    k = (wi - 2) // 2 - 8   # 4, 5, 6
                    _normalize(pcB[0], 128 * k, 0, 1024 + 128 * k, 128)
                    _normalize(pcB[1], 128 * k, 1, 1024 + 128 * k, 128)
                    avail = 9 + k
                if wi >= 10 and nd < avail:
                    _dense_tile(nd, use_act=(nd >= 6))
                    nd += 1
            while pendq:
                _ctx(*pendq.pop(0))
            if nd < 15:
                _dense_tile(nd, use_act=True)
                nd += 1
            _normalize(pcB[0], 896, 0, 1920, 128)
            _normalize(pcB[1], 896, 1, 1920, 128)
            while nd < 16:
                _dense_tile(nd, use_act=(nd % 2 == 1))
                nd += 1
    nc.compile()
    return nc


# ----------------------------------------------------------------------------
# L3: LN2 + MLP, 4 token groups x 2-way tensor-parallel over 4H
# ----------------------------------------------------------------------------

def build_l3():
    nc = bacc.Bacc("TRN2", target_bir_lowering=False, debug=False,
                   num_devices=NCORE)
    # xres: [p, 1024t + f] = attn_out token 128t+p (512 tokens per core)
    xresD = nc.dram_tensor("xres", [128, 4 * H], BF16, kind="ExternalInput")
    # f1T: m-major packing: [p, 1024*m + 128*c + j] = fc1_eff.T[128c+p, 2048*half + 128m + j]
    f1TD = nc.dram_tensor("f1T", [4, 128, 4 * H], BF16, kind="ExternalInput")
    b1D = nc.dram_tensor("b1c", [128, 16], F32, kind="ExternalInput")
    # f2T: chunk m at cols [1024m:1024(m+1)): fc2_w.T[2048*half+128m+p, f]
    f2TD = nc.dram_tensor("f2T", [4, 128, 4 * H], BF16, kind="ExternalInput")
    outP = nc.dram_tensor("outP", [128, 4 * H], BF16, kind="ExternalOutput")

    with tile.TileContext(nc) as tc, ExitStack() as ctx:
        singles = ctx.enter_context(tc.tile_pool(name="singles", bufs=1))
        stat = ctx.enter_context(tc.tile_pool(name="stat", bufs=4))
        work = ctx.enter_context(tc.tile_pool(name="work", bufs=3))
        hp = ctx.enter_context(tc.tile_pool(name="hp", bufs=1))
        outp = ctx.enter_context(tc.tile_pool(name="outp", bufs=3))

        ident = singles.tile([128, 128], BF16)
        make_identity(nc, ident)
        epst = singles.tile([128, 1], F32)
        nc.vector.memset(epst, EPS)
        warm = singles.tile([1, 1], F32)
        nc.scalar.activation(out=warm, in_=epst[0:1, 0:1],
                             func=mybir.ActivationFunctionType.Sqrt,
                             bias=0.0, scale=1.0)

        xres = singles.tile([128, 4, H], BF16)
        b1c = singles.tile([128, 16], F32)
        f1 = [singles.tile([128, 4 * H], BF16, tag=f"f1{g}", name=f"f1{g}")
              for g in range(4)]
        f2 = [singles.tile([128, 4 * H], BF16, tag=f"f2{g}", name=f"f2{g}")
              for g in range(4)]
        # interleave: first two xres tiles (gate LN2 for fc1's first half),
        # then the first fc1 group, then the rest
        nc.sync.dma_start(out=xres[:, 0, :], in_=xresD.ap()[:, 0:H])
        nc.sync.dma_start(out=xres[:, 1, :], in_=xresD.ap()[:, H:2 * H])
        nc.sync.dma_start(out=b1c, in_=b1D.ap())
        nc.sync.dma_start(out=f1[0], in_=f1TD.ap()[0])
        nc.sync.dma_start(out=xres[:, 2, :], in_=xresD.ap()[:, 2 * H:3 * H])
        nc.sync.dma_start(out=xres[:, 3, :], in_=xresD.ap()[:, 3 * H:4 * H])
        for g in range(1, 4):
            nc.sync.dma_start(out=f1[g], in_=f1TD.ap()[g])
        for g in range(4):
            nc.sync.dma_start(out=f2[g], in_=f2TD.ap()[g])

        # xh2T: [p, c, 128t+j] = xhat^T chunk c
        xh2T = singles.tile([128, NHC, 512], BF16)

        ones_pe = singles.tile([1, 512], BF16)
        nc.vector.memset(ones_pe, 1.0)
        with tc.tile_pool(name="ps", bufs=1, space="PSUM") as pp:
            pwarm = pp.tile([128, 512], F32, tag="f1", bufs=3, name="pwarm")
            for _ in range(9):
                nc.tensor.matmul(pwarm, lhsT=ones_pe[0:1, 0:128],
                                 rhs=ones_pe[0:1, 0:512],
                                 start=True, stop=True)
            for t in range(4):
                st = stat.tile([128, 2, 6], F32, tag="bnst")
                nc.vector.bn_stats(out=st[:, 0, :], in_=xres[:, t, 0:512])
                nc.vector.bn_stats(out=st[:, 1, :], in_=xres[:, t, 512:1024])
                mv = stat.tile([128, 2], F32, tag="bnmv")
                nc.vector.bn_aggr(out=mv, in_=st)
                rstd = stat.tile([128, 1], F32, tag="rstd")
                nc.scalar.activation(out=rstd, in_=mv[:, 1:2],
                                     func=mybir.ActivationFunctionType.Sqrt,
                                     bias=epst, scale=1.0)
                nc.vector.reciprocal(out=rstd, in_=rstd)
                xh = work.tile([128, H], BF16, tag="xhat")
                nc.vector.tensor_scalar(out=xh, in0=xres[:, t, :],
                                        scalar1=mv[:, 0:1], scalar2=rstd,
                                        op0=mybir.AluOpType.subtract,
                                        op1=mybir.AluOpType.mult)
                tp = pp.tile([128, 1024], BF16, tag="tp", bufs=1)
                for c in range(NHC):
                    nc.tensor.transpose(tp[:, c * 128:(c + 1) * 128],
                                        xh[:, c * 128:(c + 1) * 128], ident)
                nc.vector.tensor_copy(out=xh2T[:, :, t * 128:(t + 1) * 128],
                                      in_=tp)

            hts = {}

            def _fc1(m):
                ps = pp.tile([128, 512], F32, tag="f1", bufs=3, name="psf1")
                # token-quarters: the first fc1 only waits on LN2 of tile 0
                for tg in range(4):
                    for c in range(NHC):
                        nc.tensor.matmul(
                            ps[:, tg * 128:(tg + 1) * 128],
                            lhsT=f1[m // 4][:, (m % 4) * 1024 + c * 128:
                                            (m % 4) * 1024 + (c + 1) * 128],
                            rhs=xh2T[:, c, tg * 128:(tg + 1) * 128],
                            start=(c == 0), stop=(c == NHC - 1))
                ht = hp.tile([128, 512], BF16, tag=f"h{m}")
                if os.environ.get("BLOOM_SIM"):
                    u = work.tile([128, 512], F32, tag="gelu_u")
                    nc.vector.tensor_scalar_add(out=u, in0=ps,
                                                scalar1=b1c[:, m:m + 1])
                    s2 = work.tile([128, 512], F32, tag="gelu_s")
                    nc.vector.tensor_mul(out=s2, in0=u, in1=u)
                    nc.vector.tensor_scalar(out=s2, in0=s2,
                                            scalar1=0.035677408145115,
                                            scalar2=0.7978845608028654,
                                            op0=mybir.AluOpType.mult,
                                            op1=mybir.AluOpType.add)
                    nc.vector.tensor_mul(out=s2, in0=s2, in1=u)
                    nc.scalar.activation(out=s2, in_=s2,
                                         func=mybir.ActivationFunctionType.Tanh,
                                         bias=0.0, scale=1.0)
                    nc.vector.tensor_scalar(out=s2, in0=s2, scalar1=1.0,
                                            scalar2=0.5,
                                            op0=mybir.AluOpType.add,
                                            op1=mybir.AluOpType.mult)
                    nc.vector.tensor_mul(out=ht, in0=s2, in1=u)
                else:
                    nc.scalar.activation(
                        out=ht, in_=ps,
                        func=mybir.ActivationFunctionType.Gelu_apprx_tanh,
                        bias=b1c[:, m:m + 1], scale=1.0)
                hts[m] = ht

            def _fc2(psf2, m, tpair):
                ht = hts[m]
                for ti, t in enumerate(tpair):
                    for n in range(2):
                        nc.tensor.matmul(
                            psf2[ti][:, n * 512:(n + 1) * 512],
                            lhsT=ht[:, t * 128:(t + 1) * 128],
                            rhs=f2[m // 4][:, (m % 4) * 1024 + n * 512:
                                           (m % 4) * 1024 + (n + 1) * 512],
                            start=(m == 0), stop=(m == 15))

            def _drain(psf2, tpair):
                for ti, t in enumerate(tpair):
                    stg = outp.tile([128, 1024], BF16, tag="og")
                    if t % 2 == 0:
                        nc.vector.tensor_copy(out=stg, in_=psf2[ti])
                    else:
                        nc.scalar.activation(
                            out=stg, in_=psf2[ti],
                            func=mybir.ActivationFunctionType.Copy,
                            bias=0.0, scale=1.0)
                    nc.sync.dma_start(out=outP.ap()[:, t * H:(t + 1) * H],
                                      in_=stg)

            # pass 1: fc1 all m, fc2 into token tiles 0,1 (staggered)
            psf2 = [pp.tile([128, 1024], F32, tag=f"f2_{t}", bufs=1,
                            name=f"psf2_{t}")
                    for t in range(2)]
            _fc1(0)
            for m in range(16):
                if m + 1 < 16:
                    _fc1(m + 1)
                _fc2(psf2, m, (0, 1))
            _drain(psf2, (0, 1))
            # passes 2/3: fc2 for token tiles 2 then 3 (staggered drains)
            psf2b = [pp.tile([128, 1024], F32, tag="f2_0", bufs=1,
                             name="psf2b")]
            for m in range(16):
                _fc2(psf2b, m, (2,))
            _drain(psf2b, (2,))
            psf2c = [pp.tile([128, 1024], F32, tag="f2_1", bufs=1,
                             name="psf2c")]
            for m in range(16):
                _fc2(psf2c, m, (3,))
            _drain(psf2c, (3,))
    nc.compile()
    return nc


# ----------------------------------------------------------------------------
# host orchestration
# ----------------------------------------------------------------------------

_NC_CACHE = {}
_BUILDERS = {"l1": build_l1, "l2": build_l2, "l3": build_l3}


def _get_nc(name):
    if name not in _NC_CACHE:
        _NC_CACHE[name] = _BUILDERS[name]()
    return _NC_CACHE[name]


def _run(nc, in_maps):
    if os.environ.get("BLOOM_SIM"):
        from concourse.bass_interp import CoreSim
        results = []
        for m in in_maps:
            sim = CoreSim(nc, trace=False)
            for k, v in m.items():
                sim.tensor(k)[:] = v
            sim.simulate(check_with_hw=False)
            outs = {}
            for alloc in nc.m.functions[0].allocations:
                if getattr(alloc, "kind", None) == "ExternalOutput":
                    nm = alloc.memorylocations[0].name
                    outs[nm] = np.array(sim.tensor(nm))
            results.append(outs)
        return results
    from concourse.bass_utils import run_bass_kernel_spmd
    res = run_bass_kernel_spmd(nc, in_maps, core_ids=list(range(NCORE)))
    return res.results


def _prep_weights(ln1_g, ln1_b, qkv_w, qkv_b, dense_w, dense_b,
                  ln2_g, ln2_b, fc1_w, fc1_b, fc2_w, fc2_b):
    qkv_w = np.asarray(qkv_w, np.float32)
    qkv_b = np.asarray(qkv_b, np.float32)
    w_eff = qkv_w * np.asarray(ln1_g, np.float32)[None, :]
    b_eff = qkv_b + qkv_w @ np.asarray(ln1_b, np.float32)
    w3 = w_eff.reshape(NH, 3 * HD, H)
    b3 = b_eff.reshape(NH, 3 * HD)
    wq = w3[:, :HD, :] / NORM
    wk = w3[:, HD:2 * HD, :]
    wv = w3[:, 2 * HD:, :]
    bq = b3[:, :HD] / NORM
    bk = b3[:, HD:2 * HD]
    bv = b3[:, 2 * HD:]

    def pack_lhsT(w_pair):
        """[128 f, 1024 h] -> [p, 128c + f] = w_pair[f, 128c + p]"""
        return np.ascontiguousarray(
            w_pair.T.reshape(NHC, 128, 128).transpose(1, 0, 2)
            .reshape(128, NHC * 128)).astype(NBF)

    wQ_i, wK_i, wV_i, bcol_i, bvr_i = [], [], [], [], []
    for i in range(NCORE):
        h0, h1 = 2 * i, 2 * i + 1
        wQ_i.append(pack_lhsT(np.concatenate([wq[h0], wq[h1]], 0)))
        wK_i.append(pack_lhsT(np.concatenate([wk[h0], wk[h1]], 0)))
        wV_i.append(pack_lhsT(np.concatenate([wv[h0], wv[h1]], 0)))
        bcol_i.append(np.concatenate(
            [bq[h0], bq[h1], bk[h0], bk[h1]]).reshape(1, 256).astype(NBF))
        bvr_i.append(np.concatenate([bv[h0], bv[h1]])
                     .reshape(1, 128).astype(NBF))

    dwT = np.ascontiguousarray(np.asarray(dense_w, np.float32).T).astype(NBF)
    db_r = np.asarray(dense_b, np.float32).reshape(1, H)

    f1_eff = np.asarray(fc1_w, np.float32) * np.asarray(ln2_g, np.float32)[None, :]
    b1_eff = np.asarray(fc1_b, np.float32) + np.asarray(fc1_w, np.float32) @ np.asarray(ln2_b, np.float32)
    fc1T = np.ascontiguousarray(f1_eff.T)                           # [H, 4H]
    fc2T = np.ascontiguousarray(np.asarray(fc2_w, np.float32).T)    # [4H, H]
    b2_r = np.asarray(fc2_b, np.float32).reshape(1, H)

    # L3 packings, per half
    f1T_half, b1_half, f2T_half = [], [], []
    for half in range(2):
        cols = slice(half * 2 * H, (half + 1) * 2 * H)
        f1h = fc1T[:, cols]                                         # [1024, 2048]
        # f1TD[g, p, 1024*(m%4) + 128c + j] = f1h[128c + p, 128m + j]
        a = (f1h.reshape(NHC, 128, 16, 128)      # [c, p, m, j]
             .transpose(2, 1, 0, 3)              # [m, p, c, j]
             .reshape(4, 4, 128, NHC * 128)      # [g, m%4, p, c*j]
             .transpose(0, 2, 1, 3)              # [g, p, m%4, c*j]
             .reshape(4, 128, 4 * H))
        f1T_half.append(np.ascontiguousarray(a.astype(NBF)))
        b1h = b1_eff[half * 2 * H:(half + 1) * 2 * H]
        b1_half.append(np.ascontiguousarray(
            b1h.reshape(16, 128).T).astype(np.float32))
        f2h = fc2T[half * 2 * H:(half + 1) * 2 * H, :]              # [2048, 1024]
        # f2TD[g, p, 1024*(m%4) + f] = f2h[128m + p, f]
        b = (f2h.reshape(4, 4, 128, H)           # [g, m%4, p, f]
             .transpose(0, 2, 1, 3)              # [g, p, m%4, f]
             .reshape(4, 128, 4 * H))
        f2T_half.append(np.ascontiguousarray(b.astype(NBF)))
    return dict(wQ=wQ_i, wK=wK_i, wV=wV_i, bcol=bcol_i, bvr=bvr_i, db=db_r,
                dwT=dwT, f1T_half=f1T_half, b1_half=b1_half,
                f2T_half=f2T_half, b2=b2_r)


def _tri_mask():
    k = np.arange(QB)[:, None]
    q = np.arange(QB)[None, :]
    return np.where(k <= q, 1.0, 0.0).astype(NBF)   # [k, q] allowed k<=q


def kernel(hidden_states, attention_mask, alibi,
           ln1_g, ln1_b, qkv_w, qkv_b, dense_w, dense_b,
           ln2_g, ln2_b, fc1_w, fc1_b, fc2_w, fc2_b):
    X = np.asarray(hidden_states, np.float32).reshape(S, H)
    alibi_np = np.asarray(alibi, np.float32).reshape(NH, S)
    W = _prep_weights(ln1_g, ln1_b, qkv_w, qkv_b, dense_w, dense_b,
                      ln2_g, ln2_b, fc1_w, fc1_b, fc2_w, fc2_b)

    # ---------------- L1: LN1 + transpose ----------------
    nc1 = _get_nc("l1")
    in1 = []
    for i in range(NCORE):
        a, b = _blocks(i)
        xi = np.concatenate([X[a * QB:(a + 1) * QB], X[b * QB:(b + 1) * QB]], 0)
        xp = np.ascontiguousarray(
            xi.reshape(2, 128, H).transpose(1, 0, 2)).astype(NBF)
        in1.append(dict(x=xp))
    r1 = _run(nc1, in1)

    # ---------------- host gather: xhat^T chunk-major ----------------
    xhG = np.zeros((NHC, 128, S), NBF)
    for i in range(NCORE):
        a, b = _blocks(i)
        r = r1[i]["xhT"].reshape(2, 128, NHC, 128)
        xhG[:, :, a * QB:(a + 1) * QB] = r[0].transpose(1, 0, 2)
        xhG[:, :, b * QB:(b + 1) * QB] = r[1].transpose(1, 0, 2)
    xhG = np.ascontiguousarray(xhG)

    # ---------------- L2: QKV + attention + dense partial ----------------
    binm = _tri_mask()
    dwT = W["dwT"]
    nc2 = _get_nc("l2")
    in2 = []
    for i in range(NCORE):
        alc = np.zeros((128, 32), np.float32)
        for j in range(2):
            alc[:, 16 * j:16 * (j + 1)] = \
                alibi_np[2 * i + j].reshape(16, 128).T
        dwi = np.ascontiguousarray(dwT[i * 128:(i + 1) * 128, :]).astype(NBF)
        in2.append(dict(xh=xhG, wQ=W["wQ"][i], wK=W["wK"][i], wV=W["wV"][i],
                        bw=W["bcol"][i], bvr=W["bvr"][i], alc=alc,
                        binm=binm, dw=dwi))
    r2 = _run(nc2, in2)

    # host reduce: attn_out = sum of dense partials + residual + dense bias
    attn_out = X + W["db"]
    for i in range(NCORE):
        attn_out = attn_out + r2[i]["outD"].astype(np.float32) \
            .reshape(128, 16, H).transpose(1, 0, 2).reshape(S, H)

    # ---------------- L3 ----------------
    nc3 = _get_nc("l3")
    in3 = []
    attn_bf = attn_out.astype(NBF)
    for i in range(NCORE):
        g, half = i // 2, i % 2
        xg = attn_bf[512 * g:512 * (g + 1)]        # [512, H]
        xres = np.ascontiguousarray(
            xg.reshape(4, 128, H).transpose(1, 0, 2).reshape(128, 4 * H))
        in3.append(dict(xres=xres, f1T=W["f1T_half"][half],
                        b1c=W["b1_half"][half], f2T=W["f2T_half"][half]))
    r3 = _run(nc3, in3)

    out = np.empty((S, H), np.float32)
    for g in range(4):
        p = r3[2 * g]["outP"].astype(np.float32) + \
            r3[2 * g + 1]["outP"].astype(np.float32)
        out[512 * g:512 * (g + 1)] = \
            p.reshape(128, 4, H).transpose(1, 0, 2).reshape(512, H) \
            + attn_out[512 * g:512 * (g + 1)] + W["b2"]
    return out.reshape(1, S, H)


# revision 65
# speedup vs baseline: 1.0021x; 1.0012x over previous
"""BloomBlock on 8 TRN2 NeuronCores — 3-launch structure.

  * L1 (data-parallel over tokens): LN1 (folded into weights on host) +
    QKV projection for each core's 256 tokens (blocks i and 15-i).
  * Host: all-gather Q/K/V, regroup per head.
  * L2 (tensor-parallel over heads): each core owns 2 heads for ALL 2048
    queries. Exact-causal attention (no padded key slots): per key chunk
    c, only queries >= 128c are scored. Transposed-score layout (keys on
    partitions, queries on free dim; softmax denominator via an appended
    ones-column on V; alibi via a bias row on K matched with a ones row
    on Q). Diagonal chunks get a post-exp binary stair mask. Fused
    row-parallel dense: each core emits a partial dense output over all
    tokens from its 2 heads' context.
  * Host: reduce dense partials + residual + dense bias -> attn_out.
  * L3 (4 token groups x 2-way tensor-parallel MLP): each core runs LN2
    on its group's 512 tokens and computes fc1/gelu/fc2 for half the 4H
    features; partial fc2 outputs are reduced on host with residual2.
"""

import os
from contextlib import ExitStack

import ml_dtypes
import numpy as np

import concourse.bass as bass
import concourse.tile as tile
from concourse import bacc, mybir
from concourse.masks import make_identity

BF16 = mybir.dt.bfloat16
F32 = mybir.dt.float32
NBF = ml_dtypes.bfloat16

S, H, NH, HD = 2048, 1024, 16, 64
NCORE = 8
QB = 128          # token/key chunk size
SC = 2 * QB       # tokens per core in L1
NSLOT = 16
NHC = H // 128    # hidden chunks
EPS = 1e-5
NORM = float(np.sqrt(HD))  # 8.0 (LAYER_NUMBER = 1)


def _blocks(i):
    return (i, 15 - i)


# ----------------------------------------------------------------------------
# L1: LN1 + QKV, data-parallel over tokens (unchanged from baseline)
# ----------------------------------------------------------------------------

def build_l1():
    """Tiny launch: LN1 + transpose only (token-DP, 256 tokens/core)."""
    nc = bacc.Bacc("TRN2", target_bir_lowering=False, debug=False,
                   num_devices=NCORE)
    # x token-major packed: [p, t, f] = token 128t+p (bf16)
    x = nc.dram_tensor("x", [128, 2, H], BF16, kind="ExternalInput")
    # xhT: [t, p, 128c + j] = xhat^T[128c+p, token 128t+j]
    xhT = nc.dram_tensor("xhT", [2, 128, NHC * 128], BF16,
                         kind="ExternalOutput")

    with tile.TileContext(nc) as tc, ExitStack() as ctx:
        singles = ctx.enter_context(tc.tile_pool(name="singles", bufs=1))
        stat = ctx.enter_context(tc.tile_pool(name="stat", bufs=4))
        work = ctx.enter_context(tc.tile_pool(name="work", bufs=3))

        ident = singles.tile([128, 128], BF16)
        make_identity(nc, ident)
        epst = singles.tile([128, 1], F32)
        nc.vector.memset(epst, EPS)
        warm = singles.tile([1, 1], F32)
        nc.scalar.activation(out=warm, in_=epst[0:1, 0:1],
                             func=mybir.ActivationFunctionType.Sqrt,
                             bias=0.0, scale=1.0)
        xt = singles.tile([128, 2, H], BF16)
        nc.sync.dma_start(out=xt[:, 0, :], in_=x.ap()[:, 0, :])
        nc.sync.dma_start(out=xt[:, 1, :], in_=x.ap()[:, 1, :])
        stage = singles.tile([128, 2, NHC * 128], BF16)

        with tc.tile_pool(name="tp_ps", bufs=2, space="PSUM") as tp_ps:
            # p-state warm-up (overwritten by the real transposes)
            pw = tp_ps.tile([128, 512], F32, tag="pw", bufs=1, name="pw")
            for _ in range(20):
                nc.tensor.matmul(pw[:, 0:128], lhsT=ident[0:1, :],
                                 rhs=ident[0:1, :], start=True, stop=True)
            # both stat chains first so the DVE stream never gates t=1
            rstds = []
            for t in range(2):
                st = stat.tile([128, 2, 6], F32, tag="bnst")
                nc.vector.bn_stats(out=st[:, 0, :], in_=xt[:, t, 0:512])
                nc.vector.bn_stats(out=st[:, 1, :], in_=xt[:, t, 512:1024])
                mv = stat.tile([128, 2], F32, tag="bnmv")
                nc.vector.bn_aggr(out=mv, in_=st)
                rstd = stat.tile([128, 1], F32, tag="rstd")
                nc.scalar.activation(out=rstd, in_=mv[:, 1:2],
                                     func=mybir.ActivationFunctionType.Sqrt,
                                     bias=epst, scale=1.0)
                nc.vector.reciprocal(out=rstd, in_=rstd)
                rstds.append((mv, rstd))
            for t in range(2):
                mv, rstd = rstds[t]
                xh = work.tile([128, H], BF16, tag="xhat")
                nc.vector.tensor_scalar(out=xh, in0=xt[:, t, :],
                                        scalar1=mv[:, 0:1], scalar2=rstd,
                                        op0=mybir.AluOpType.subtract,
                                        op1=mybir.AluOpType.mult)
                tp = tp_ps.tile([128, NHC * 128], BF16, tag="tp")
                for c in range(NHC):
                    nc.tensor.transpose(tp[:, c * 128:(c + 1) * 128],
                                        xh[:, c * 128:(c + 1) * 128], ident)
                nc.vector.tensor_copy(out=stage[:, t, :], in_=tp)
                nc.sync.dma_start(out=xhT.ap()[t], in_=stage[:, t, :])
    nc.compile()
    return nc


# ----------------------------------------------------------------------------
# L2: exact-causal attention, tensor-parallel over heads (2 heads/core),
#     fused row-parallel dense partial.
# ----------------------------------------------------------------------------

def build_l2():
    """QKV (2 heads/core over all tokens) + exact-causal attention +
    row-parallel dense partial."""
    nc = bacc.Bacc("TRN2", target_bir_lowering=False, debug=False,
                   num_devices=NCORE)
    # xhat^T chunk-major (replicated input): [c, p, tok]
    xhD = nc.dram_tensor("xh", [NHC, 128, S], BF16, kind="ExternalInput")
    # lhsT weight packs, chunk c at cols [128c,128c+128): [q0|q1], [k0|k1]
    wQD = nc.dram_tensor("wQ", [128, NHC * 128], BF16, kind="ExternalInput")
    wKD = nc.dram_tensor("wK", [128, NHC * 128], BF16, kind="ExternalInput")
    # v rhs pack, chunk c: [128 h, [v0|v1] 128]
    wVD = nc.dram_tensor("wV", [128, NHC * 128], BF16, kind="ExternalInput")
    bwD = nc.dram_tensor("bw", [1, 256], BF16, kind="ExternalInput")
    bvrD = nc.dram_tensor("bvr", [1, 128], BF16, kind="ExternalInput")
    # alibi columns: [p, 16h + c] = alibi[head h, key 128c+p]
    alcD = nc.dram_tensor("alc", [128, 32], F32, kind="ExternalInput")
    binmD = nc.dram_tensor("binm", [QB, QB], BF16, kind="ExternalInput")
    dwD = nc.dram_tensor("dw", [128, H], BF16, kind="ExternalInput")
    # dense partial, token-major packed: [p, 1024*t + f] = token 128t+p
    outD = nc.dram_tensor("outD", [128, 16 * H], BF16, kind="ExternalOutput")

    with tile.TileContext(nc) as tc, ExitStack() as ctx:
        singles = ctx.enter_context(tc.tile_pool(name="singles", bufs=1))
        probs = ctx.enter_context(tc.tile_pool(name="probs", bufs=18))
        work = ctx.enter_context(tc.tile_pool(name="work", bufs=4))
        outp = ctx.enter_context(tc.tile_pool(name="outp", bufs=6))

        binm = singles.tile([QB, QB], BF16)
        dw = singles.tile([128, H], BF16)
        ctxT = singles.tile([128, S], BF16)
        qaugP = singles.tile([128, S], BF16)
        kaugP = singles.tile([128, S], BF16)
        alc_sb = singles.tile([128, 32], F32)
        bw_sb = singles.tile([1, 256], BF16)
        bv_sb = singles.tile([1, 128], BF16)
        ones_row = singles.tile([1, 1024], BF16)
        nc.vector.memset(ones_row, 1.0)
        vaug = []
        for h in range(2):
            va = singles.tile([128, 16, 65], BF16, tag=f"vaug{h}",
                              name=f"vaug{h}")
            nc.vector.memset(va, 1.0)
            vaug.append(va)
        epst = singles.tile([1, 1], F32)
        nc.vector.memset(epst, EPS)
        warm = singles.tile([1, 1], F32)
        nc.scalar.activation(out=warm, in_=epst,
                             func=mybir.ActivationFunctionType.Exp,
                             bias=0.0, scale=1.0)

        wQ = singles.tile([128, NHC * 128], BF16)
        nc.sync.dma_start(out=wQ, in_=wQD.ap())
        wK = singles.tile([128, NHC * 128], BF16)
        nc.sync.dma_start(out=wK, in_=wKD.ap())
        xh = []
        for c in range(NHC):
            xc = singles.tile([128, S], BF16, tag=f"xh{c}", name=f"xh{c}")
            nc.sync.dma_start(out=xc, in_=xhD.ap()[c])
            xh.append(xc)
        wV = singles.tile([128, NHC * 128], BF16)
        nc.sync.dma_start(out=wV, in_=wVD.ap())
        nc.sync.dma_start(out=bw_sb, in_=bwD.ap())
        nc.sync.dma_start(out=bv_sb, in_=bvrD.ap())
        nc.sync.dma_start(out=alc_sb, in_=alcD.ap())
        nc.sync.dma_start(out=binm, in_=binmD.ap())
        nc.sync.dma_start(out=dw, in_=dwD.ap())

        # ---------------- P1: Q/K projections (column halves) ------------
        # both halves accumulate interleaved; half 0 ([0:1024), all the A
        # sweep needs) drains first so attention exps start ~8us earlier
        with tc.tile_pool(name="qkv_ps", bufs=1, space="PSUM") as pqk:
            psQh = [pqk.tile([128, 1024], F32, tag=f"pq{hf}", bufs=1,
                             name=f"psQ{hf}") for hf in range(2)]
            psKh = [pqk.tile([128, 1024], F32, tag=f"pk{hf}", bufs=1,
                             name=f"psK{hf}") for hf in range(2)]
            # p-state warm-up: keep PE continuously busy from t~0.8us so the
            # real matmuls dispatch at full clock (results are overwritten by
            # the first start=True accumulation below)
            for _ in range(7):
                nc.tensor.matmul(psQh[0][:, 0:512],
                                 lhsT=ones_row[0:1, 0:128],
                                 rhs=ones_row[0:1, 0:512],
                                 start=True, stop=True)
            for c in range(NHC):
                for hf in range(2):
                    for n in range(2):
                        nc.tensor.matmul(
                            psQh[hf][:, n * 512:(n + 1) * 512],
                            lhsT=wQ[:, c * 128:(c + 1) * 128],
                            rhs=xh[c][:, hf * 1024 + n * 512:
                                       hf * 1024 + (n + 1) * 512],
                            start=(c == 0), stop=False)
                for hf in range(2):
                    for n in range(2):
                        nc.tensor.matmul(
                            psKh[hf][:, n * 512:(n + 1) * 512],
                            lhsT=wK[:, c * 128:(c + 1) * 128],
                            rhs=xh[c][:, hf * 1024 + n * 512:
                                       hf * 1024 + (n + 1) * 512],
                            start=(c == 0), stop=False)
            # bias as rank-1 updates so the drains become plain copies
            # that split across DVE and Act in parallel
            for hf in range(2):
                for n in range(2):
                    nc.tensor.matmul(
                        psQh[hf][:, n * 512:(n + 1) * 512],
                        lhsT=bw_sb[0:1, 0:128],
                        rhs=ones_row[:, 0:512],
                        start=False, stop=True)
                    nc.tensor.matmul(
                        psKh[hf][:, n * 512:(n + 1) * 512],
                        lhsT=bw_sb[0:1, 128:256],
                        rhs=ones_row[:, 0:512],
                        start=False, stop=True)
            for hf in range(2):
                lo = hf * 1024
                nc.vector.tensor_copy(out=qaugP[:, lo:lo + 1024],
                                      in_=psQh[hf])
                nc.scalar.activation(out=kaugP[:, lo:lo + 1024],
                                     in_=psKh[hf],
                                     func=mybir.ActivationFunctionType.Copy,
                                     bias=0.0, scale=1.0)

        # ---------------- P2: A-sweep scores/exp + V, interleaved ----------
        # A-sweep probs are saved to dedicated tiles; their ctx matmuls are
        # deferred to P3 where they fill PE slack under the B-sweep.
        probsA = ctx.enter_context(tc.tile_pool(name="probsA", bufs=1))
        pA = {}
        with tc.tile_pool(name="ps", bufs=1, space="PSUM") as pp:
            vps = [pp.tile([128, 8, 128], F32, tag="vp", bufs=2,
                           name=f"vp{b}") for b in range(2)]
            for wi, (h, c) in enumerate([(h, c) for c in range(8)
                                         for h in range(2)]):
                base = 128 * c
                n = 1024 - base
                p0 = 64 * h
                ps = pp.tile([128, 1024], F32, tag="sc", bufs=2, name="psA")
                for s in range(0, n, 512):
                    sl = min(512, n - s)
                    nc.tensor.matmul(
                        ps[:, s:s + sl],
                        lhsT=kaugP[p0:p0 + 64, base:base + 128],
                        rhs=qaugP[p0:p0 + 64, base + s:base + s + sl],
                        start=True, stop=True)
                pb = probsA.tile([128, 1024], BF16, tag=f"pA{wi}",
                                 name=f"pA{wi}")
                col = 16 * h + c
                nc.scalar.activation(out=pb[:, 0:n], in_=ps[:, 0:n],
                                     func=mybir.ActivationFunctionType.Exp,
                                     bias=alc_sb[:, col:col + 1], scale=1.0)
                nc.vector.tensor_mul(out=pb[:, 0:128], in0=pb[:, 0:128],
                                     in1=binm)
                pA[(h, c)] = pb
                # one V token-tile per window (PE filler under Act exps)
                t = wi
                b, ti = t // 8, t % 8
                for cc in range(NHC):
                    nc.tensor.matmul(
                        vps[b][:, ti, :],
                        lhsT=xh[cc][:, t * 128:(t + 1) * 128],
                        rhs=wV[:, cc * 128:(cc + 1) * 128],
                        start=(cc == 0), stop=False)
                nc.tensor.matmul(vps[b][:, ti, :], lhsT=ones_row[:, 0:128],
                                 rhs=bv_sb, start=False, stop=True)
                if ti == 7:
                    for h2 in range(2):
                        nc.vector.tensor_copy(
                            out=vaug[h2][:, 8 * b:8 * (b + 1), 0:64],
                            in_=vps[b][:, :, 64 * h2:64 * h2 + 64])

            # ------------- P3: B-sweep + deferred A-ctx + dense -------------

            def _dense_tile(t, use_act=False):
                psd = pp.tile([128, 1024], F32, tag="sc", bufs=2, name="psd")
                for n in range(2):
                    nc.tensor.matmul(
                        psd[:, n * 512:(n + 1) * 512],
                        lhsT=ctxT[:, t * 128:(t + 1) * 128],
                        rhs=dw[:, n * 512:(n + 1) * 512],
                        start=True, stop=True)
                stg = outp.tile([128, 1024], BF16, tag="og")
                if use_act:
                    nc.scalar.activation(
                        out=stg, in_=psd,
                        func=mybir.ActivationFunctionType.Copy,
                        bias=0.0, scale=1.0)
                else:
                    nc.vector.tensor_copy(out=stg, in_=psd)
                nc.sync.dma_start(out=outD.ap()[:, t * H:(t + 1) * H],
                                  in_=stg)

            def _scores(h, c, qa, qb):
                base = 128 * c
                n = qb - qa
                p0 = 64 * h
                ps = pp.tile([128, 1024], F32, tag="sc", bufs=2, name="ps")
                for s in range(0, n, 512):
                    sl = min(512, n - s)
                    nc.tensor.matmul(
                        ps[:, s:s + sl],
                        lhsT=kaugP[p0:p0 + 64, base:base + 128],
                        rhs=qaugP[p0:p0 + 64, qa + s:qa + s + sl],
                        start=True, stop=True)
                pb = probs.tile([128, 1024], BF16, tag="pb")
                col = 16 * h + c
                nc.scalar.activation(out=pb[:, 0:n], in_=ps[:, 0:n],
                                     func=mybir.ActivationFunctionType.Exp,
                                     bias=alc_sb[:, col:col + 1], scale=1.0)
                if qa == base:
                    nc.vector.tensor_mul(out=pb[:, 0:128], in0=pb[:, 0:128],
                                         in1=binm)
                return pb

            def _ctx(h, pc, c, qa, qb, pb):
                n = qb - qa
                slices = []
                if qa == 128 * c:
                    slices.append((0, 128, True))
                    p = 128
                else:
                    p = 0
                while p < n:
                    nxt = min(n, ((qa + p) // 512 + 1) * 512 - qa)
                    slices.append((p, nxt - p, False))
                    p = nxt
                off = qa if qa < 1024 else qa - 1024
                for (s, sl, stp) in slices:
                    o = off + s
                    nc.tensor.matmul(pc[0:65, o // 128:(o + sl) // 128, :],
                                     lhsT=vaug[h][:, c, 0:65],
                                     rhs=pb[:, s:s + sl],
                                     start=(c == 0), stop=stp,
                                     skip_group_check=True)

            def _normalize(pc, src0, h, dst0, n):
                a, b2 = src0 // 128, (src0 + n) // 128
                rec = work.tile([1, 1024], F32, tag="rec")
                nc.vector.reciprocal(out=rec[:, 0:n], in_=pc[64:65, a:b2, :])
                recb = work.tile([64, 1024], F32, tag="recb")
                nc.gpsimd.partition_broadcast(recb[:, 0:n], rec[0:1, 0:n],
                                              channels=64)
                nc.vector.tensor_mul(
                    out=ctxT[64 * h:64 * h + 64, dst0:dst0 + n],
                    in0=pc[0:64, a:b2, :], in1=recb[:, 0:n])

            pcA = [pp.tile([128, 8, 128], F32, tag="vp", bufs=2,
                           name=f"pcA{h}") for h in range(2)]
            pcB = [pp.tile([128, 8, 128], F32, tag="vp", bufs=2,
                           name=f"pcB{h}") for h in range(2)]
            winsB = [(h, c, (1024 if c < 8 else 128 * c), 2048)
                     for c in range(16) for h in range(2)]
            pendq = []
            actx = [(h, c) for c in range(8) for h in range(2)]
            nd = 0
            avail = 0
            na = 0
            for wi, (h, c, qa, qb) in enumerate(winsB):
                pb = _scores(h, c, qa, qb)
                pendq.append((h, pcB[h], c, qa, qb, pb))
                # A-ctx fillers: two per window over the first 8 windows
                if wi < 8:
                    for _ in range(2):
                        ah, ac = actx[na]
                        _ctx(ah, pcA[ah], ac, 128 * ac, 1024, pA[(ah, ac)])
                        na += 1
                # B-ctx: deferred until pcA is consumed (its psum banks are
                # reused by pcB); catch up 3 per window
                if wi >= 12:
                    for _ in range(4):
                        if len(pendq) > 1:
                            _ctx(*pendq.pop(0))
                if wi == 8:
                    _normalize(pcA[0], 0, 0, 0, 1024)
                    _normalize(pcA[1], 0, 1, 0, 1024)
                    avail = 8
                if wi == 24:
                    _normalize(pcB[0], 0, 0, 1024, 512)
                    _normalize(pcB[1], 0, 1, 1024, 512)
                    avail = 12
                if wi in (26, 28, 30):
                    k = (wi - 2) // 2 - 8   # 4, 5, 6
                    _normalize(pcB[0], 128 * k, 0, 1024 + 128 * k, 128)
                    _normalize(pcB[1], 128 * k, 1, 1024 + 128 * k, 128)
                    avail = 9 + k
                if wi >= 10 and nd < avail:
                    _dense_tile(nd, use_act=(nd >= 6))
                    nd += 1
            while pendq:
                _ctx(*pendq.pop(0))
            if nd < 15:
                _dense_tile(nd, use_act=True)
                nd += 1
            _normalize(pcB[0], 896, 0, 1920, 128)
            _normalize(pcB[1], 896, 1, 1920, 128)
            while nd < 16:
                _dense_tile(nd, use_act=(nd % 2 == 1))
                nd += 1
    nc.compile()
    return nc


# ----------------------------------------------------------------------------
# L3: LN2 + MLP, 4 token groups x 2-way tensor-parallel over 4H
# ----------------------------------------------------------------------------

def build_l3():
    nc = bacc.Bacc("TRN2", target_bir_lowering=False, debug=False,
                   num_devices=NCORE)
    # xres: [p, 1024t + f] = attn_out token 128t+p (512 tokens per core)
    xresD = nc.dram_tensor("xres", [128, 4 * H], BF16, kind="ExternalInput")
    # f1T: m-major packing: [p, 1024*m + 128*c + j] = fc1_eff.T[128c+p, 2048*half + 128m + j]
    f1TD = nc.dram_tensor("f1T", [4, 128, 4 * H], BF16, kind="ExternalInput")
    b1D = nc.dram_tensor("b1c", [128, 16], F32, kind="ExternalInput")
    # f2T: chunk m at cols [1024m:1024(m+1)): fc2_w.T[2048*half+128m+p, f]
    f2TD = nc.dram_tensor("f2T", [4, 128, 4 * H], BF16, kind="ExternalInput")
    outP = nc.dram_tensor("outP", [128, 4 * H], BF16, kind="ExternalOutput")

    with tile.TileContext(nc) as tc, ExitStack() as ctx:
        singles = ctx.enter_context(tc.tile_pool(name="singles", bufs=1))
        stat = ctx.enter_context(tc.tile_pool(name="stat", bufs=6))
        work = ctx.enter_context(tc.tile_pool(name="work", bufs=4))
        hp = ctx.enter_context(tc.tile_pool(name="hp", bufs=1))
        outp = ctx.enter_context(tc.tile_pool(name="outp", bufs=4))

        ident = singles.tile([128, 128], BF16)
        make_identity(nc, ident)
        epst = singles.tile([128, 1], F32)
        nc.vector.memset(epst, EPS)
        warm = singles.tile([1, 1], F32)
        nc.scalar.activation(out=warm, in_=epst[0:1, 0:1],
                             func=mybir.ActivationFunctionType.Sqrt,
                             bias=0.0, scale=1.0)

        xres = singles.tile([128, 4, H], BF16)
        b1c = singles.tile([128, 16], F32)
        f1 = [singles.tile([128, 4 * H], BF16, tag=f"f1{g}", name=f"f1{g}")
              for g in range(4)]
        f2 = [singles.tile([128, 4 * H], BF16, tag=f"f2{g}", name=f"f2{g}")
              for g in range(4)]
        # interleave: first two xres tiles (gate LN2 for fc1's first half),
        # then the first fc1 group, then the rest
        nc.sync.dma_start(out=xres[:, 0, :], in_=xresD.ap()[:, 0:H])
        nc.sync.dma_start(out=xres[:, 1, :], in_=xresD.ap()[:, H:2 * H])
        nc.sync.dma_start(out=b1c, in_=b1D.ap())
        nc.sync.dma_start(out=f1[0], in_=f1TD.ap()[0])
        nc.sync.dma_start(out=xres[:, 2, :], in_=xresD.ap()[:, 2 * H:3 * H])
        nc.sync.dma_start(out=xres[:, 3, :], in_=xresD.ap()[:, 3 * H:4 * H])
        for g in range(1, 4):
            nc.sync.dma_start(out=f1[g], in_=f1TD.ap()[g])
        for g in range(4):
            nc.sync.dma_start(out=f2[g], in_=f2TD.ap()[g])

        # xh2T: [p, c, 128t+j] = xhat^T chunk c
        xh2T = singles.tile([128, NHC, 512], BF16)

        ones_pe = singles.tile([1, 512], BF16)
        nc.vector.memset(ones_pe, 1.0)
        with tc.tile_pool(name="ps", bufs=1, space="PSUM") as pp:
            pwarm = pp.tile([128, 512], F32, tag="f1", bufs=3, name="pwarm")
            for _ in range(9):
                nc.tensor.matmul(pwarm, lhsT=ones_pe[0:1, 0:128],
                                 rhs=ones_pe[0:1, 0:512],
                                 start=True, stop=True)
            for t in range(4):
                st = stat.tile([128, 2, 6], F32, tag="bnst")
                nc.vector.bn_stats(out=st[:, 0, :], in_=xres[:, t, 0:512])
                nc.vector.bn_stats(out=st[:, 1, :], in_=xres[:, t, 512:1024])
                mv = stat.tile([128, 2], F32, tag="bnmv")
                nc.vector.bn_aggr(out=mv, in_=st)
                rstd = stat.tile([128, 1], F32, tag="rstd")
                nc.scalar.activation(out=rstd, in_=mv[:, 1:2],
                                     func=mybir.ActivationFunctionType.Sqrt,
                                     bias=epst, scale=1.0)
                nc.vector.reciprocal(out=rstd, in_=rstd)
                xh = work.tile([128, H], BF16, tag="xhat")
                nc.vector.tensor_scalar(out=xh, in0=xres[:, t, :],
                                        scalar1=mv[:, 0:1], scalar2=rstd,
                                        op0=mybir.AluOpType.subtract,
                                        op1=mybir.AluOpType.mult)
                tp = pp.tile([128, 1024], BF16, tag="tp", bufs=1)
                for c in range(NHC):
                    nc.tensor.transpose(tp[:, c * 128:(c + 1) * 128],
                                        xh[:, c * 128:(c + 1) * 128], ident)
                nc.vector.tensor_copy(out=xh2T[:, :, t * 128:(t + 1) * 128],
                                      in_=tp)

            hts = {}

            def _fc1(m):
                ps = pp.tile([128, 512], F32, tag="f1", bufs=3, name="psf1")
                # token-quarters: the first fc1 only waits on LN2 of tile 0
                for tg in range(4):
                    for c in range(NHC):
                        nc.tensor.matmul(
                            ps[:, tg * 128:(tg + 1) * 128],
                            lhsT=f1[m // 4][:, (m % 4) * 1024 + c * 128:
                                            (m % 4) * 1024 + (c + 1) * 128],
                            rhs=xh2T[:, c, tg * 128:(tg + 1) * 128],
                            start=(c == 0), stop=(c == NHC - 1))
                ht = hp.tile([128, 512], BF16, tag=f"h{m}")
                if os.environ.get("BLOOM_SIM"):
                    u = work.tile([128, 512], F32, tag="gelu_u")
                    nc.vector.tensor_scalar_add(out=u, in0=ps,
                                                scalar1=b1c[:, m:m + 1])
                    s2 = work.tile([128, 512], F32, tag="gelu_s")
                    nc.vector.tensor_mul(out=s2, in0=u, in1=u)
                    nc.vector.tensor_scalar(out=s2, in0=s2,
                                            scalar1=0.035677408145115,
                                            scalar2=0.7978845608028654,
                                            op0=mybir.AluOpType.mult,
                                            op1=mybir.AluOpType.add)
                    nc.vector.tensor_mul(out=s2, in0=s2, in1=u)
                    nc.scalar.activation(out=s2, in_=s2,
                                         func=mybir.ActivationFunctionType.Tanh,
                                         bias=0.0, scale=1.0)
                    nc.vector.tensor_scalar(out=s2, in0=s2, scalar1=1.0,
                                            scalar2=0.5,
                                            op0=mybir.AluOpType.add,
                                            op1=mybir.AluOpType.mult)
                    nc.vector.tensor_mul(out=ht, in0=s2, in1=u)
                else:
                    nc.scalar.activation(
                        out=ht, in_=ps,
                        func=mybir.ActivationFunctionType.Gelu_apprx_tanh,
                        bias=b1c[:, m:m + 1], scale=1.0)
                hts[m] = ht

            def _fc2(psf2, m, tpair):
                ht = hts[m]
                for ti, t in enumerate(tpair):
                    for n in range(2):
                        nc.tensor.matmul(
                            psf2[ti][:, n * 512:(n + 1) * 512],
                            lhsT=ht[:, t * 128:(t + 1) * 128],
                            rhs=f2[m // 4][:, (m % 4) * 1024 + n * 512:
                                           (m % 4) * 1024 + (n + 1) * 512],
                            start=(m == 0), stop=(m == 15))

            def _drain(psf2, tpair):
                for ti, t in enumerate(tpair):
                    stg = outp.tile([128, 1024], BF16, tag="og")
                    if t % 2 == 0:
                        nc.vector.tensor_copy(out=stg, in_=psf2[ti])
                    else:
                        nc.scalar.activation(
                            out=stg, in_=psf2[ti],
                            func=mybir.ActivationFunctionType.Copy,
                            bias=0.0, scale=1.0)
                    nc.sync.dma_start(out=outP.ap()[:, t * H:(t + 1) * H],
                                      in_=stg)

            # pass 1: fc1 all m, fc2 into token tiles 0,1 (staggered)
            psf2 = [pp.tile([128, 1024], F32, tag=f"f2_{t}", bufs=1,
                            name=f"psf2_{t}")
                    for t in range(2)]
            _fc1(0)
            for m in range(16):
                if m + 1 < 16:
                    _fc1(m + 1)
                _fc2(psf2, m, (0, 1))
            _drain(psf2, (0, 1))
            # passes 2/3: fc2 for token tiles 2 then 3 (staggered drains)
            psf2b = [pp.tile([128, 1024], F32, tag="f2_0", bufs=1,
                             name="psf2b")]
            for m in range(16):
                _fc2(psf2b, m, (2,))
            _drain(psf2b, (2,))
            psf2c = [pp.tile([128, 1024], F32, tag="f2_1", bufs=1,
                             name="psf2c")]
            for m in range(16):
                _fc2(psf2c, m, (3,))
            _drain(psf2c, (3,))
    nc.compile()
    return nc


# ----------------------------------------------------------------------------
# host orchestration
# ----------------------------------------------------------------------------

_NC_CACHE = {}
_BUILDERS = {"l1": build_l1, "l2": build_l2, "l3": build_l3}


def _get_nc(name):
    if name not in _NC_CACHE:
        _NC_CACHE[name] = _BUILDERS[name]()
    return _NC_CACHE[name]


def _run(nc, in_maps):
    if os.environ.get("BLOOM_SIM"):
        from concourse.bass_interp import CoreSim
        results = []
        for m in in_maps:
            sim = CoreSim(nc, trace=False)
            for k, v in m.items():
                sim.tensor(k)[:] = v
            sim.simulate(check_with_hw=False)
            outs = {}
            for alloc in nc.m.functions[0].allocations:
                if getattr(alloc, "kind", None) == "ExternalOutput":
                    nm = alloc.memorylocations[0].name
                    outs[nm] = np.array(sim.tensor(nm))
            results.append(outs)
        return results
    from concourse.bass_utils import run_bass_kernel_spmd
    res = run_bass_kernel_spmd(nc, in_maps, core_ids=list(range(NCORE)))
    return res.results


def _prep_weights(ln1_g, ln1_b, qkv_w, qkv_b, dense_w, dense_b,
                  ln2_g, ln2_b, fc1_w, fc1_b, fc2_w, fc2_b):
    qkv_w = np.asarray(qkv_w, np.float32)
    qkv_b = np.asarray(qkv_b, np.float32)
    w_eff = qkv_w * np.asarray(ln1_g, np.float32)[None, :]
    b_eff = qkv_b + qkv_w @ np.asarray(ln1_b, np.float32)
    w3 = w_eff.reshape(NH, 3 * HD, H)
    b3 = b_eff.reshape(NH, 3 * HD)
    wq = w3[:, :HD, :] / NORM
    wk = w3[:, HD:2 * HD, :]
    wv = w3[:, 2 * HD:, :]
    bq = b3[:, :HD] / NORM
    bk = b3[:, HD:2 * HD]
    bv = b3[:, 2 * HD:]

    def pack_lhsT(w_pair):
        """[128 f, 1024 h] -> [p, 128c + f] = w_pair[f, 128c + p]"""
        return np.ascontiguousarray(
            w_pair.T.reshape(NHC, 128, 128).transpose(1, 0, 2)
            .reshape(128, NHC * 128)).astype(NBF)

    wQ_i, wK_i, wV_i, bcol_i, bvr_i = [], [], [], [], []
    for i in range(NCORE):
        h0, h1 = 2 * i, 2 * i + 1
        wQ_i.append(pack_lhsT(np.concatenate([wq[h0], wq[h1]], 0)))
        wK_i.append(pack_lhsT(np.concatenate([wk[h0], wk[h1]], 0)))
        wV_i.append(pack_lhsT(np.concatenate([wv[h0], wv[h1]], 0)))
        bcol_i.append(np.concatenate(
            [bq[h0], bq[h1], bk[h0], bk[h1]]).reshape(1, 256).astype(NBF))
        bvr_i.append(np.concatenate([bv[h0], bv[h1]])
                     .reshape(1, 128).astype(NBF))

    dwT = np.ascontiguousarray(np.asarray(dense_w, np.float32).T).astype(NBF)
    db_r = np.asarray(dense_b, np.float32).reshape(1, H)

    f1_eff = np.asarray(fc1_w, np.float32) * np.asarray(ln2_g, np.float32)[None, :]
    b1_eff = np.asarray(fc1_b, np.float32) + np.asarray(fc1_w, np.float32) @ np.asarray(ln2_b, np.float32)
    fc1T = np.ascontiguousarray(f1_eff.T)                           # [H, 4H]
    fc2T = np.ascontiguousarray(np.asarray(fc2_w, np.float32).T)    # [4H, H]
    b2_r = np.asarray(fc2_b, np.float32).reshape(1, H)

    # L3 packings, per half
    f1T_half, b1_half, f2T_half = [], [], []
    for half in range(2):
        cols = slice(half * 2 * H, (half + 1) * 2 * H)
        f1h = fc1T[:, cols]                                         # [1024, 2048]
        # f1TD[g, p, 1024*(m%4) + 128c + j] = f1h[128c + p, 128m + j]
        a = (f1h.reshape(NHC, 128, 16, 128)      # [c, p, m, j]
             .transpose(2, 1, 0, 3)              # [m, p, c, j]
             .reshape(4, 4, 128, NHC * 128)      # [g, m%4, p, c*j]
             .transpose(0, 2, 1, 3)              # [g, p, m%4, c*j]
             .reshape(4, 128, 4 * H))
        f1T_half.append(np.ascontiguousarray(a.astype(NBF)))
        b1h = b1_eff[half * 2 * H:(half + 1) * 2 * H]
        b1_half.append(np.ascontiguousarray(
            b1h.reshape(16, 128).T).astype(np.float32))
        f2h = fc2T[half * 2 * H:(half + 1) * 2 * H, :]              # [2048, 1024]
        # f2TD[g, p, 1024*(m%4) + f] = f2h[128m + p, f]
        b = (f2h.reshape(4, 4, 128, H)           # [g, m%4, p, f]
             .transpose(0, 2, 1, 3)              # [g, p, m%4, f]
             .reshape(4, 128, 4 * H))
        f2T_half.append(np.ascontiguousarray(b.astype(NBF)))
    return dict(wQ=wQ_i, wK=wK_i, wV=wV_i, bcol=bcol_i, bvr=bvr_i, db=db_r,
                dwT=dwT, f1T_half=f1T_half, b1_half=b1_half,
                f2T_half=f2T_half, b2=b2_r)


def _tri_mask():
    k = np.arange(QB)[:, None]
    q = np.arange(QB)[None, :]
    return np.where(k <= q, 1.0, 0.0).astype(NBF)   # [k, q] allowed k<=q


def kernel(hidden_states, attention_mask, alibi,
           ln1_g, ln1_b, qkv_w, qkv_b, dense_w, dense_b,
           ln2_g, ln2_b, fc1_w, fc1_b, fc2_w, fc2_b):
    X = np.asarray(hidden_states, np.float32).reshape(S, H)
    alibi_np = np.asarray(alibi, np.float32).reshape(NH, S)
    W = _prep_weights(ln1_g, ln1_b, qkv_w, qkv_b, dense_w, dense_b,
                      ln2_g, ln2_b, fc1_w, fc1_b, fc2_w, fc2_b)

    # ---------------- L1: LN1 + transpose ----------------
    nc1 = _get_nc("l1")
    in1 = []
    for i in range(NCORE):
        a, b = _blocks(i)
        xi = np.concatenate([X[a * QB:(a + 1) * QB], X[b * QB:(b + 1) * QB]], 0)
        xp = np.ascontiguousarray(
            xi.reshape(2, 128, H).transpose(1, 0, 2)).astype(NBF)
        in1.append(dict(x=xp))
    r1 = _run(nc1, in1)

    # ---------------- host gather: xhat^T chunk-major ----------------
    xhG = np.zeros((NHC, 128, S), NBF)
    for i in range(NCORE):
        a, b = _blocks(i)
        r = r1[i]["xhT"].reshape(2, 128, NHC, 128)
        xhG[:, :, a * QB:(a + 1) * QB] = r[0].transpose(1, 0, 2)
        xhG[:, :, b * QB:(b + 1) * QB] = r[1].transpose(1, 0, 2)
    xhG = np.ascontiguousarray(xhG)

    # ---------------- L2: QKV + attention + dense partial ----------------
    binm = _tri_mask()
    dwT = W["dwT"]
    nc2 = _get_nc("l2")
    in2 = []
    for i in range(NCORE):
        alc = np.zeros((128, 32), np.float32)
        for j in range(2):
            alc[:, 16 * j:16 * (j + 1)] = \
                alibi_np[2 * i + j].reshape(16, 128).T
        dwi = np.ascontiguousarray(dwT[i * 128:(i + 1) * 128, :]).astype(NBF)
        in2.append(dict(xh=xhG, wQ=W["wQ"][i], wK=W["wK"][i], wV=W["wV"][i],
                        bw=W["bcol"][i], bvr=W["bvr"][i], alc=alc,
                        binm=binm, dw=dwi))
    r2 = _run(nc2, in2)

    # host reduce: attn_out = sum of dense partials + residual + dense bias
    attn_out = X + W["db"]
    for i in range(NCORE):
        attn_out = attn_out + r2[i]["outD"].astype(np.float32) \
            .reshape(128, 16, H).transpose(1, 0, 2).reshape(S, H)

    # ---------------- L3 ----------------
    nc3 = _get_nc("l3")
    in3 = []
    attn_bf = attn_out.astype(NBF)
    for i in range(NCORE):
        g, half = i // 2, i % 2
        xg = attn_bf[512 * g:512 * (g + 1)]        # [512, H]
        xres = np.ascontiguousarray(
            xg.reshape(4, 128, H).transpose(1, 0, 2).reshape(128, 4 * H))
        in3.append(dict(xres=xres, f1T=W["f1T_half"][half],
                        b1c=W["b1_half"][half], f2T=W["f2T_half"][half]))
    r3 = _run(nc3, in3)

    out = np.empty((S, H), np.float32)
    for g in range(4):
        p = r3[2 * g]["outP"].astype(np.float32) + \
            r3[2 * g + 1]["outP"].astype(np.float32)
        out[512 * g:512 * (g + 1)] = \
            p.reshape(128, 4, H).transpose(1, 0, 2).reshape(512, H) \
            + attn_out[512 * g:512 * (g + 1)] + W["b2"]
    return out.reshape(1, S, H)


# revision 66
# speedup vs baseline: 1.0026x; 1.0004x over previous
"""BloomBlock on 8 TRN2 NeuronCores — 3-launch structure.

  * L1 (data-parallel over tokens): LN1 (folded into weights on host) +
    QKV projection for each core's 256 tokens (blocks i and 15-i).
  * Host: all-gather Q/K/V, regroup per head.
  * L2 (tensor-parallel over heads): each core owns 2 heads for ALL 2048
    queries. Exact-causal attention (no padded key slots): per key chunk
    c, only queries >= 128c are scored. Transposed-score layout (keys on
    partitions, queries on free dim; softmax denominator via an appended
    ones-column on V; alibi via a bias row on K matched with a ones row
    on Q). Diagonal chunks get a post-exp binary stair mask. Fused
    row-parallel dense: each core emits a partial dense output over all
    tokens from its 2 heads' context.
  * Host: reduce dense partials + residual + dense bias -> attn_out.
  * L3 (4 token groups x 2-way tensor-parallel MLP): each core runs LN2
    on its group's 512 tokens and computes fc1/gelu/fc2 for half the 4H
    features; partial fc2 outputs are reduced on host with residual2.
"""

import os
from contextlib import ExitStack

import ml_dtypes
import numpy as np

import concourse.bass as bass
import concourse.tile as tile
from concourse import bacc, mybir
from concourse.masks import make_identity

BF16 = mybir.dt.bfloat16
F32 = mybir.dt.float32
NBF = ml_dtypes.bfloat16

S, H, NH, HD = 2048, 1024, 16, 64
NCORE = 8
QB = 128          # token/key chunk size
SC = 2 * QB       # tokens per core in L1
NSLOT = 16
NHC = H // 128    # hidden chunks
EPS = 1e-5
NORM = float(np.sqrt(HD))  # 8.0 (LAYER_NUMBER = 1)


def _blocks(i):
    return (i, 15 - i)


# ----------------------------------------------------------------------------
# L1: LN1 + QKV, data-parallel over tokens (unchanged from baseline)
# ----------------------------------------------------------------------------

def build_l1():
    """Tiny launch: LN1 + transpose only (token-DP, 256 tokens/core)."""
    nc = bacc.Bacc("TRN2", target_bir_lowering=False, debug=False,
                   num_devices=NCORE)
    # x token-major packed: [p, t, f] = token 128t+p (bf16)
    x = nc.dram_tensor("x", [128, 2, H], BF16, kind="ExternalInput")
    # xhT: [t, p, 128c + j] = xhat^T[128c+p, token 128t+j]
    xhT = nc.dram_tensor("xhT", [2, 128, NHC * 128], BF16,
                         kind="ExternalOutput")

    with tile.TileContext(nc) as tc, ExitStack() as ctx:
        singles = ctx.enter_context(tc.tile_pool(name="singles", bufs=1))
        stat = ctx.enter_context(tc.tile_pool(name="stat", bufs=4))
        work = ctx.enter_context(tc.tile_pool(name="work", bufs=3))

        ident = singles.tile([128, 128], BF16)
        make_identity(nc, ident)
        epst = singles.tile([128, 1], F32)
        nc.vector.memset(epst, EPS)
        warm = singles.tile([1, 1], F32)
        nc.scalar.activation(out=warm, in_=epst[0:1, 0:1],
                             func=mybir.ActivationFunctionType.Sqrt,
                             bias=0.0, scale=1.0)
        xt = singles.tile([128, 2, H], BF16)
        nc.sync.dma_start(out=xt[:, 0, :], in_=x.ap()[:, 0, :])
        nc.sync.dma_start(out=xt[:, 1, :], in_=x.ap()[:, 1, :])
        stage = singles.tile([128, 2, NHC * 128], BF16)

        with tc.tile_pool(name="tp_ps", bufs=2, space="PSUM") as tp_ps:
            # p-state warm-up (overwritten by the real transposes)
            pw = tp_ps.tile([128, 512], F32, tag="pw", bufs=1, name="pw")
            for _ in range(20):
                nc.tensor.matmul(pw[:, 0:128], lhsT=ident[0:1, :],
                                 rhs=ident[0:1, :], start=True, stop=True)
            # both stat chains first so the DVE stream never gates t=1
            rstds = []
            for t in range(2):
                st = stat.tile([128, 2, 6], F32, tag="bnst")
                nc.vector.bn_stats(out=st[:, 0, :], in_=xt[:, t, 0:512])
                nc.vector.bn_stats(out=st[:, 1, :], in_=xt[:, t, 512:1024])
                mv = stat.tile([128, 2], F32, tag="bnmv")
                nc.vector.bn_aggr(out=mv, in_=st)
                rstd = stat.tile([128, 1], F32, tag="rstd")
                nc.scalar.activation(out=rstd, in_=mv[:, 1:2],
                                     func=mybir.ActivationFunctionType.Sqrt,
                                     bias=epst, scale=1.0)
                nc.vector.reciprocal(out=rstd, in_=rstd)
                rstds.append((mv, rstd))
            for t in range(2):
                mv, rstd = rstds[t]
                xh = work.tile([128, H], BF16, tag="xhat")
                nc.vector.tensor_scalar(out=xh, in0=xt[:, t, :],
                                        scalar1=mv[:, 0:1], scalar2=rstd,
                                        op0=mybir.AluOpType.subtract,
                                        op1=mybir.AluOpType.mult)
                tp = tp_ps.tile([128, NHC * 128], BF16, tag="tp")
                for c in range(NHC):
                    nc.tensor.transpose(tp[:, c * 128:(c + 1) * 128],
                                        xh[:, c * 128:(c + 1) * 128], ident)
                nc.vector.tensor_copy(out=stage[:, t, :], in_=tp)
                nc.sync.dma_start(out=xhT.ap()[t], in_=stage[:, t, :])
    nc.compile()
    return nc


# ----------------------------------------------------------------------------
# L2: exact-causal attention, tensor-parallel over heads (2 heads/core),
#     fused row-parallel dense partial.
# ----------------------------------------------------------------------------

def build_l2():
    """QKV (2 heads/core over all tokens) + exact-causal attention +
    row-parallel dense partial."""
    nc = bacc.Bacc("TRN2", target_bir_lowering=False, debug=False,
                   num_devices=NCORE)
    # xhat^T chunk-major (replicated input): [c, p, tok]
    xhD = nc.dram_tensor("xh", [NHC, 128, S], BF16, kind="ExternalInput")
    # lhsT weight packs, chunk c at cols [128c,128c+128): [q0|q1], [k0|k1]
    wQD = nc.dram_tensor("wQ", [128, NHC * 128], BF16, kind="ExternalInput")
    wKD = nc.dram_tensor("wK", [128, NHC * 128], BF16, kind="ExternalInput")
    # v rhs pack, chunk c: [128 h, [v0|v1] 128]
    wVD = nc.dram_tensor("wV", [128, NHC * 128], BF16, kind="ExternalInput")
    bwD = nc.dram_tensor("bw", [1, 256], BF16, kind="ExternalInput")
    bvrD = nc.dram_tensor("bvr", [1, 128], BF16, kind="ExternalInput")
    # alibi columns: [p, 16h + c] = alibi[head h, key 128c+p]
    alcD = nc.dram_tensor("alc", [128, 32], F32, kind="ExternalInput")
    binmD = nc.dram_tensor("binm", [QB, QB], BF16, kind="ExternalInput")
    dwD = nc.dram_tensor("dw", [128, H], BF16, kind="ExternalInput")
    # dense partial, token-major packed: [p, 1024*t + f] = token 128t+p
    outD = nc.dram_tensor("outD", [128, 16 * H], BF16, kind="ExternalOutput")

    with tile.TileContext(nc) as tc, ExitStack() as ctx:
        singles = ctx.enter_context(tc.tile_pool(name="singles", bufs=1))
        probs = ctx.enter_context(tc.tile_pool(name="probs", bufs=20))
        work = ctx.enter_context(tc.tile_pool(name="work", bufs=5))
        outp = ctx.enter_context(tc.tile_pool(name="outp", bufs=8))

        binm = singles.tile([QB, QB], BF16)
        dw = singles.tile([128, H], BF16)
        ctxT = singles.tile([128, S], BF16)
        qaugP = singles.tile([128, S], BF16)
        kaugP = singles.tile([128, S], BF16)
        alc_sb = singles.tile([128, 32], F32)
        bw_sb = singles.tile([1, 256], BF16)
        bv_sb = singles.tile([1, 128], BF16)
        ones_row = singles.tile([1, 1024], BF16)
        nc.vector.memset(ones_row, 1.0)
        vaug = []
        for h in range(2):
            va = singles.tile([128, 16, 65], BF16, tag=f"vaug{h}",
                              name=f"vaug{h}")
            nc.vector.memset(va, 1.0)
            vaug.append(va)
        epst = singles.tile([1, 1], F32)
        nc.vector.memset(epst, EPS)
        warm = singles.tile([1, 1], F32)
        nc.scalar.activation(out=warm, in_=epst,
                             func=mybir.ActivationFunctionType.Exp,
                             bias=0.0, scale=1.0)

        wQ = singles.tile([128, NHC * 128], BF16)
        nc.sync.dma_start(out=wQ, in_=wQD.ap())
        wK = singles.tile([128, NHC * 128], BF16)
        nc.sync.dma_start(out=wK, in_=wKD.ap())
        xh = []
        for c in range(NHC):
            xc = singles.tile([128, S], BF16, tag=f"xh{c}", name=f"xh{c}")
            nc.sync.dma_start(out=xc, in_=xhD.ap()[c])
            xh.append(xc)
        wV = singles.tile([128, NHC * 128], BF16)
        nc.sync.dma_start(out=wV, in_=wVD.ap())
        nc.sync.dma_start(out=bw_sb, in_=bwD.ap())
        nc.sync.dma_start(out=bv_sb, in_=bvrD.ap())
        nc.sync.dma_start(out=alc_sb, in_=alcD.ap())
        nc.sync.dma_start(out=binm, in_=binmD.ap())
        nc.sync.dma_start(out=dw, in_=dwD.ap())

        # ---------------- P1: Q/K projections (column halves) ------------
        # both halves accumulate interleaved; half 0 ([0:1024), all the A
        # sweep needs) drains first so attention exps start ~8us earlier
        with tc.tile_pool(name="qkv_ps", bufs=1, space="PSUM") as pqk:
            psQh = [pqk.tile([128, 1024], F32, tag=f"pq{hf}", bufs=1,
                             name=f"psQ{hf}") for hf in range(2)]
            psKh = [pqk.tile([128, 1024], F32, tag=f"pk{hf}", bufs=1,
                             name=f"psK{hf}") for hf in range(2)]
            # p-state warm-up: keep PE continuously busy from t~0.8us so the
            # real matmuls dispatch at full clock (results are overwritten by
            # the first start=True accumulation below)
            for _ in range(7):
                nc.tensor.matmul(psQh[0][:, 0:512],
                                 lhsT=ones_row[0:1, 0:128],
                                 rhs=ones_row[0:1, 0:512],
                                 start=True, stop=True)
            for c in range(NHC):
                for hf in range(2):
                    for n in range(2):
                        nc.tensor.matmul(
                            psQh[hf][:, n * 512:(n + 1) * 512],
                            lhsT=wQ[:, c * 128:(c + 1) * 128],
                            rhs=xh[c][:, hf * 1024 + n * 512:
                                       hf * 1024 + (n + 1) * 512],
                            start=(c == 0), stop=False)
                for hf in range(2):
                    for n in range(2):
                        nc.tensor.matmul(
                            psKh[hf][:, n * 512:(n + 1) * 512],
                            lhsT=wK[:, c * 128:(c + 1) * 128],
                            rhs=xh[c][:, hf * 1024 + n * 512:
                                       hf * 1024 + (n + 1) * 512],
                            start=(c == 0), stop=False)
            # bias as rank-1 updates so the drains become plain copies
            # that split across DVE and Act in parallel
            for hf in range(2):
                for n in range(2):
                    nc.tensor.matmul(
                        psQh[hf][:, n * 512:(n + 1) * 512],
                        lhsT=bw_sb[0:1, 0:128],
                        rhs=ones_row[:, 0:512],
                        start=False, stop=True)
                    nc.tensor.matmul(
                        psKh[hf][:, n * 512:(n + 1) * 512],
                        lhsT=bw_sb[0:1, 128:256],
                        rhs=ones_row[:, 0:512],
                        start=False, stop=True)
            for hf in range(2):
                lo = hf * 1024
                nc.vector.tensor_copy(out=qaugP[:, lo:lo + 1024],
                                      in_=psQh[hf])
                nc.scalar.activation(out=kaugP[:, lo:lo + 1024],
                                     in_=psKh[hf],
                                     func=mybir.ActivationFunctionType.Copy,
                                     bias=0.0, scale=1.0)

        # ---------------- P2: A-sweep scores/exp + V, interleaved ----------
        # A-sweep probs are saved to dedicated tiles; their ctx matmuls are
        # deferred to P3 where they fill PE slack under the B-sweep.
        probsA = ctx.enter_context(tc.tile_pool(name="probsA", bufs=1))
        pA = {}
        with tc.tile_pool(name="ps", bufs=1, space="PSUM") as pp:
            vps = [pp.tile([128, 8, 128], F32, tag="vp", bufs=2,
                           name=f"vp{b}") for b in range(2)]
            for wi, (h, c) in enumerate([(h, c) for c in range(8)
                                         for h in range(2)]):
                base = 128 * c
                n = 1024 - base
                p0 = 64 * h
                ps = pp.tile([128, 1024], F32, tag="sc", bufs=2, name="psA")
                for s in range(0, n, 512):
                    sl = min(512, n - s)
                    nc.tensor.matmul(
                        ps[:, s:s + sl],
                        lhsT=kaugP[p0:p0 + 64, base:base + 128],
                        rhs=qaugP[p0:p0 + 64, base + s:base + s + sl],
                        start=True, stop=True)
                pb = probsA.tile([128, 1024], BF16, tag=f"pA{wi}",
                                 name=f"pA{wi}")
                col = 16 * h + c
                nc.scalar.activation(out=pb[:, 0:n], in_=ps[:, 0:n],
                                     func=mybir.ActivationFunctionType.Exp,
                                     bias=alc_sb[:, col:col + 1], scale=1.0)
                nc.vector.tensor_mul(out=pb[:, 0:128], in0=pb[:, 0:128],
                                     in1=binm)
                pA[(h, c)] = pb
                # one V token-tile per window (PE filler under Act exps)
                t = wi
                b, ti = t // 8, t % 8
                for cc in range(NHC):
                    nc.tensor.matmul(
                        vps[b][:, ti, :],
                        lhsT=xh[cc][:, t * 128:(t + 1) * 128],
                        rhs=wV[:, cc * 128:(cc + 1) * 128],
                        start=(cc == 0), stop=False)
                nc.tensor.matmul(vps[b][:, ti, :], lhsT=ones_row[:, 0:128],
                                 rhs=bv_sb, start=False, stop=True)
                if ti == 7:
                    for h2 in range(2):
                        nc.vector.tensor_copy(
                            out=vaug[h2][:, 8 * b:8 * (b + 1), 0:64],
                            in_=vps[b][:, :, 64 * h2:64 * h2 + 64])

            # ------------- P3: B-sweep + deferred A-ctx + dense -------------

            def _dense_tile(t, use_act=False):
                psd = pp.tile([128, 1024], F32, tag="sc", bufs=2, name="psd")
                for n in range(2):
                    nc.tensor.matmul(
                        psd[:, n * 512:(n + 1) * 512],
                        lhsT=ctxT[:, t * 128:(t + 1) * 128],
                        rhs=dw[:, n * 512:(n + 1) * 512],
                        start=True, stop=True)
                stg = outp.tile([128, 1024], BF16, tag="og")
                if use_act:
                    nc.scalar.activation(
                        out=stg, in_=psd,
                        func=mybir.ActivationFunctionType.Copy,
                        bias=0.0, scale=1.0)
                else:
                    nc.vector.tensor_copy(out=stg, in_=psd)
                nc.sync.dma_start(out=outD.ap()[:, t * H:(t + 1) * H],
                                  in_=stg)

            def _scores(h, c, qa, qb):
                base = 128 * c
                n = qb - qa
                p0 = 64 * h
                ps = pp.tile([128, 1024], F32, tag="sc", bufs=2, name="ps")
                for s in range(0, n, 512):
                    sl = min(512, n - s)
                    nc.tensor.matmul(
                        ps[:, s:s + sl],
                        lhsT=kaugP[p0:p0 + 64, base:base + 128],
                        rhs=qaugP[p0:p0 + 64, qa + s:qa + s + sl],
                        start=True, stop=True)
                pb = probs.tile([128, 1024], BF16, tag="pb")
                col = 16 * h + c
                nc.scalar.activation(out=pb[:, 0:n], in_=ps[:, 0:n],
                                     func=mybir.ActivationFunctionType.Exp,
                                     bias=alc_sb[:, col:col + 1], scale=1.0)
                if qa == base:
                    nc.vector.tensor_mul(out=pb[:, 0:128], in0=pb[:, 0:128],
                                         in1=binm)
                return pb

            def _ctx(h, pc, c, qa, qb, pb):
                n = qb - qa
                slices = []
                if qa == 128 * c:
                    slices.append((0, 128, True))
                    p = 128
                else:
                    p = 0
                while p < n:
                    nxt = min(n, ((qa + p) // 512 + 1) * 512 - qa)
                    slices.append((p, nxt - p, False))
                    p = nxt
                off = qa if qa < 1024 else qa - 1024
                for (s, sl, stp) in slices:
                    o = off + s
                    nc.tensor.matmul(pc[0:65, o // 128:(o + sl) // 128, :],
                                     lhsT=vaug[h][:, c, 0:65],
                                     rhs=pb[:, s:s + sl],
                                     start=(c == 0), stop=stp,
                                     skip_group_check=True)

            def _normalize(pc, src0, h, dst0, n):
                a, b2 = src0 // 128, (src0 + n) // 128
                rec = work.tile([1, 1024], F32, tag="rec")
                nc.vector.reciprocal(out=rec[:, 0:n], in_=pc[64:65, a:b2, :])
                recb = work.tile([64, 1024], F32, tag="recb")
                nc.gpsimd.partition_broadcast(recb[:, 0:n], rec[0:1, 0:n],
                                              channels=64)
                nc.vector.tensor_mul(
                    out=ctxT[64 * h:64 * h + 64, dst0:dst0 + n],
                    in0=pc[0:64, a:b2, :], in1=recb[:, 0:n])

            pcA = [pp.tile([128, 8, 128], F32, tag="vp", bufs=2,
                           name=f"pcA{h}") for h in range(2)]
            pcB = [pp.tile([128, 8, 128], F32, tag="vp", bufs=2,
                           name=f"pcB{h}") for h in range(2)]
            winsB = [(h, c, (1024 if c < 8 else 128 * c), 2048)
                     for c in range(16) for h in range(2)]
            pendq = []
            actx = [(h, c) for c in range(8) for h in range(2)]
            nd = 0
            avail = 0
            na = 0
            for wi, (h, c, qa, qb) in enumerate(winsB):
                pb = _scores(h, c, qa, qb)
                pendq.append((h, pcB[h], c, qa, qb, pb))
                # A-ctx fillers: two per window over the first 8 windows
                if wi < 8:
                    for _ in range(2):
                        ah, ac = actx[na]
                        _ctx(ah, pcA[ah], ac, 128 * ac, 1024, pA[(ah, ac)])
                        na += 1
                # B-ctx: deferred until pcA is consumed (its psum banks are
                # reused by pcB); catch up 3 per window
                if wi >= 12:
                    for _ in range(4):
                        if len(pendq) > 1:
                            _ctx(*pendq.pop(0))
                if wi == 8:
                    _normalize(pcA[0], 0, 0, 0, 1024)
                    _normalize(pcA[1], 0, 1, 0, 1024)
                    avail = 8
                if wi == 24:
                    _normalize(pcB[0], 0, 0, 1024, 512)
                    _normalize(pcB[1], 0, 1, 1024, 512)
                    avail = 12
                if wi in (26, 28, 30):
                    k = (wi - 2) // 2 - 8   # 4, 5, 6
                    _normalize(pcB[0], 128 * k, 0, 1024 + 128 * k, 128)
                    _normalize(pcB[1], 128 * k, 1, 1024 + 128 * k, 128)
                    avail = 9 + k
                if wi >= 10 and nd < avail:
                    _dense_tile(nd, use_act=(nd >= 6))
                    nd += 1
            while pendq:
                _ctx(*pendq.pop(0))
            if nd < 15:
                _dense_tile(nd, use_act=True)
                nd += 1
            _normalize(pcB[0], 896, 0, 1920, 128)
            _normalize(pcB[1], 896, 1, 1920, 128)
            while nd < 16:
                _dense_tile(nd, use_act=(nd % 2 == 1))
                nd += 1
    nc.compile()
    return nc


# ----------------------------------------------------------------------------
# L3: LN2 + MLP, 4 token groups x 2-way tensor-parallel over 4H
# ----------------------------------------------------------------------------

def build_l3():
    nc = bacc.Bacc("TRN2", target_bir_lowering=False, debug=False,
                   num_devices=NCORE)
    # xres: [p, 1024t + f] = attn_out token 128t+p (512 tokens per core)
    xresD = nc.dram_tensor("xres", [128, 4 * H], BF16, kind="ExternalInput")
    # f1T: m-major packing: [p, 1024*m + 128*c + j] = fc1_eff.T[128c+p, 2048*half + 128m + j]
    f1TD = nc.dram_tensor("f1T", [4, 128, 4 * H], BF16, kind="ExternalInput")
    b1D = nc.dram_tensor("b1c", [128, 16], F32, kind="ExternalInput")
    # f2T: chunk m at cols [1024m:1024(m+1)): fc2_w.T[2048*half+128m+p, f]
    f2TD = nc.dram_tensor("f2T", [4, 128, 4 * H], BF16, kind="ExternalInput")
    outP = nc.dram_tensor("outP", [128, 4 * H], BF16, kind="ExternalOutput")

    with tile.TileContext(nc) as tc, ExitStack() as ctx:
        singles = ctx.enter_context(tc.tile_pool(name="singles", bufs=1))
        stat = ctx.enter_context(tc.tile_pool(name="stat", bufs=6))
        work = ctx.enter_context(tc.tile_pool(name="work", bufs=4))
        hp = ctx.enter_context(tc.tile_pool(name="hp", bufs=1))
        outp = ctx.enter_context(tc.tile_pool(name="outp", bufs=4))

        ident = singles.tile([128, 128], BF16)
        make_identity(nc, ident)
        epst = singles.tile([128, 1], F32)
        nc.vector.memset(epst, EPS)
        warm = singles.tile([1, 1], F32)
        nc.scalar.activation(out=warm, in_=epst[0:1, 0:1],
                             func=mybir.ActivationFunctionType.Sqrt,
                             bias=0.0, scale=1.0)

        xres = singles.tile([128, 4, H], BF16)
        b1c = singles.tile([128, 16], F32)
        f1 = [singles.tile([128, 4 * H], BF16, tag=f"f1{g}", name=f"f1{g}")
              for g in range(4)]
        f2 = [singles.tile([128, 4 * H], BF16, tag=f"f2{g}", name=f"f2{g}")
              for g in range(4)]
        # interleave: first two xres tiles (gate LN2 for fc1's first half),
        # then the first fc1 group, then the rest
        nc.sync.dma_start(out=xres[:, 0, :], in_=xresD.ap()[:, 0:H])
        nc.sync.dma_start(out=xres[:, 1, :], in_=xresD.ap()[:, H:2 * H])
        nc.sync.dma_start(out=b1c, in_=b1D.ap())
        nc.sync.dma_start(out=f1[0], in_=f1TD.ap()[0])
        nc.sync.dma_start(out=xres[:, 2, :], in_=xresD.ap()[:, 2 * H:3 * H])
        nc.sync.dma_start(out=xres[:, 3, :], in_=xresD.ap()[:, 3 * H:4 * H])
        for g in range(1, 4):
            nc.sync.dma_start(out=f1[g], in_=f1TD.ap()[g])
        for g in range(4):
            nc.sync.dma_start(out=f2[g], in_=f2TD.ap()[g])

        # xh2T: [p, c, 128t+j] = xhat^T chunk c
        xh2T = singles.tile([128, NHC, 512], BF16)

        ones_pe = singles.tile([1, 512], BF16)
        nc.vector.memset(ones_pe, 1.0)
        with tc.tile_pool(name="ps", bufs=1, space="PSUM") as pp:
            pwarm = pp.tile([128, 512], F32, tag="f1", bufs=3, name="pwarm")
            for _ in range(9):
                nc.tensor.matmul(pwarm, lhsT=ones_pe[0:1, 0:128],
                                 rhs=ones_pe[0:1, 0:512],
                                 start=True, stop=True)
            for t in range(4):
                st = stat.tile([128, 2, 6], F32, tag="bnst")
                nc.vector.bn_stats(out=st[:, 0, :], in_=xres[:, t, 0:512])
                nc.vector.bn_stats(out=st[:, 1, :], in_=xres[:, t, 512:1024])
                mv = stat.tile([128, 2], F32, tag="bnmv")
                nc.vector.bn_aggr(out=mv, in_=st)
                rstd = stat.tile([128, 1], F32, tag="rstd")
                nc.scalar.activation(out=rstd, in_=mv[:, 1:2],
                                     func=mybir.ActivationFunctionType.Sqrt,
                                     bias=epst, scale=1.0)
                nc.vector.reciprocal(out=rstd, in_=rstd)
                xh = work.tile([128, H], BF16, tag="xhat")
                nc.vector.tensor_scalar(out=xh, in0=xres[:, t, :],
                                        scalar1=mv[:, 0:1], scalar2=rstd,
                                        op0=mybir.AluOpType.subtract,
                                        op1=mybir.AluOpType.mult)
                tp = pp.tile([128, 1024], BF16, tag="tp", bufs=1)
                for c in range(NHC):
                    nc.tensor.transpose(tp[:, c * 128:(c + 1) * 128],
                                        xh[:, c * 128:(c + 1) * 128], ident)
                nc.vector.tensor_copy(out=xh2T[:, :, t * 128:(t + 1) * 128],
                                      in_=tp)

            hts = {}

            def _fc1(m):
                ps = pp.tile([128, 512], F32, tag="f1", bufs=3, name="psf1")
                # token-quarters: the first fc1 only waits on LN2 of tile 0
                for tg in range(4):
                    for c in range(NHC):
                        nc.tensor.matmul(
                            ps[:, tg * 128:(tg + 1) * 128],
                            lhsT=f1[m // 4][:, (m % 4) * 1024 + c * 128:
                                            (m % 4) * 1024 + (c + 1) * 128],
                            rhs=xh2T[:, c, tg * 128:(tg + 1) * 128],
                            start=(c == 0), stop=(c == NHC - 1))
                ht = hp.tile([128, 512], BF16, tag=f"h{m}")
                if os.environ.get("BLOOM_SIM"):
                    u = work.tile([128, 512], F32, tag="gelu_u")
                    nc.vector.tensor_scalar_add(out=u, in0=ps,
                                                scalar1=b1c[:, m:m + 1])
                    s2 = work.tile([128, 512], F32, tag="gelu_s")
                    nc.vector.tensor_mul(out=s2, in0=u, in1=u)
                    nc.vector.tensor_scalar(out=s2, in0=s2,
                                            scalar1=0.035677408145115,
                                            scalar2=0.7978845608028654,
                                            op0=mybir.AluOpType.mult,
                                            op1=mybir.AluOpType.add)
                    nc.vector.tensor_mul(out=s2, in0=s2, in1=u)
                    nc.scalar.activation(out=s2, in_=s2,
                                         func=mybir.ActivationFunctionType.Tanh,
                                         bias=0.0, scale=1.0)
                    nc.vector.tensor_scalar(out=s2, in0=s2, scalar1=1.0,
                                            scalar2=0.5,
                                            op0=mybir.AluOpType.add,
                                            op1=mybir.AluOpType.mult)
                    nc.vector.tensor_mul(out=ht, in0=s2, in1=u)
                else:
                    nc.scalar.activation(
                        out=ht, in_=ps,
                        func=mybir.ActivationFunctionType.Gelu_apprx_tanh,
                        bias=b1c[:, m:m + 1], scale=1.0)
                hts[m] = ht

            def _fc2(psf2, m, tpair):
                ht = hts[m]
                for ti, t in enumerate(tpair):
                    for n in range(2):
                        nc.tensor.matmul(
                            psf2[ti][:, n * 512:(n + 1) * 512],
                            lhsT=ht[:, t * 128:(t + 1) * 128],
                            rhs=f2[m // 4][:, (m % 4) * 1024 + n * 512:
                                           (m % 4) * 1024 + (n + 1) * 512],
                            start=(m == 0), stop=(m == 15))

            def _drain(psf2, tpair):
                for ti, t in enumerate(tpair):
                    stg = outp.tile([128, 1024], BF16, tag="og")
                    if t % 2 == 0:
                        nc.vector.tensor_copy(out=stg, in_=psf2[ti])
                    else:
                        nc.scalar.activation(
                            out=stg, in_=psf2[ti],
                            func=mybir.ActivationFunctionType.Copy,
                            bias=0.0, scale=1.0)
                    nc.sync.dma_start(out=outP.ap()[:, t * H:(t + 1) * H],
                                      in_=stg)

            # pass 1: fc1 all m, fc2 into token tiles 0,1 (staggered)
            psf2 = [pp.tile([128, 1024], F32, tag=f"f2_{t}", bufs=1,
                            name=f"psf2_{t}")
                    for t in range(2)]
            _fc1(0)
            for m in range(16):
                if m + 1 < 16:
                    _fc1(m + 1)
                _fc2(psf2, m, (0, 1))
            _drain(psf2, (0, 1))
            # passes 2/3: fc2 for token tiles 2 then 3 (staggered drains)
            psf2b = [pp.tile([128, 1024], F32, tag="f2_0", bufs=1,
                             name="psf2b")]
            for m in range(16):
                _fc2(psf2b, m, (2,))
            _drain(psf2b, (2,))
            psf2c = [pp.tile([128, 1024], F32, tag="f2_1", bufs=1,
                             name="psf2c")]
            for m in range(16):
                _fc2(psf2c, m, (3,))
            _drain(psf2c, (3,))
    nc.compile()
    return nc


# ----------------------------------------------------------------------------
# host orchestration
# ----------------------------------------------------------------------------

_NC_CACHE = {}
_BUILDERS = {"l1": build_l1, "l2": build_l2, "l3": build_l3}


def _get_nc(name):
    if name not in _NC_CACHE:
        _NC_CACHE[name] = _BUILDERS[name]()
    return _NC_CACHE[name]


def _run(nc, in_maps):
    if os.environ.get("BLOOM_SIM"):
        from concourse.bass_interp import CoreSim
        results = []
        for m in in_maps:
            sim = CoreSim(nc, trace=False)
            for k, v in m.items():
                sim.tensor(k)[:] = v
            sim.simulate(check_with_hw=False)
            outs = {}
            for alloc in nc.m.functions[0].allocations:
                if getattr(alloc, "kind", None) == "ExternalOutput":
                    nm = alloc.memorylocations[0].name
                    outs[nm] = np.array(sim.tensor(nm))
            results.append(outs)
        return results
    from concourse.bass_utils import run_bass_kernel_spmd
    res = run_bass_kernel_spmd(nc, in_maps, core_ids=list(range(NCORE)))
    return res.results


def _prep_weights(ln1_g, ln1_b, qkv_w, qkv_b, dense_w, dense_b,
                  ln2_g, ln2_b, fc1_w, fc1_b, fc2_w, fc2_b):
    qkv_w = np.asarray(qkv_w, np.float32)
    qkv_b = np.asarray(qkv_b, np.float32)
    w_eff = qkv_w * np.asarray(ln1_g, np.float32)[None, :]
    b_eff = qkv_b + qkv_w @ np.asarray(ln1_b, np.float32)
    w3 = w_eff.reshape(NH, 3 * HD, H)
    b3 = b_eff.reshape(NH, 3 * HD)
    wq = w3[:, :HD, :] / NORM
    wk = w3[:, HD:2 * HD, :]
    wv = w3[:, 2 * HD:, :]
    bq = b3[:, :HD] / NORM
    bk = b3[:, HD:2 * HD]
    bv = b3[:, 2 * HD:]

    def pack_lhsT(w_pair):
        """[128 f, 1024 h] -> [p, 128c + f] = w_pair[f, 128c + p]"""
        return np.ascontiguousarray(
            w_pair.T.reshape(NHC, 128, 128).transpose(1, 0, 2)
            .reshape(128, NHC * 128)).astype(NBF)

    wQ_i, wK_i, wV_i, bcol_i, bvr_i = [], [], [], [], []
    for i in range(NCORE):
        h0, h1 = 2 * i, 2 * i + 1
        wQ_i.append(pack_lhsT(np.concatenate([wq[h0], wq[h1]], 0)))
        wK_i.append(pack_lhsT(np.concatenate([wk[h0], wk[h1]], 0)))
        wV_i.append(pack_lhsT(np.concatenate([wv[h0], wv[h1]], 0)))
        bcol_i.append(np.concatenate(
            [bq[h0], bq[h1], bk[h0], bk[h1]]).reshape(1, 256).astype(NBF))
        bvr_i.append(np.concatenate([bv[h0], bv[h1]])
                     .reshape(1, 128).astype(NBF))

    dwT = np.ascontiguousarray(np.asarray(dense_w, np.float32).T).astype(NBF)
    db_r = np.asarray(dense_b, np.float32).reshape(1, H)

    f1_eff = np.asarray(fc1_w, np.float32) * np.asarray(ln2_g, np.float32)[None, :]
    b1_eff = np.asarray(fc1_b, np.float32) + np.asarray(fc1_w, np.float32) @ np.asarray(ln2_b, np.float32)
    fc1T = np.ascontiguousarray(f1_eff.T)                           # [H, 4H]
    fc2T = np.ascontiguousarray(np.asarray(fc2_w, np.float32).T)    # [4H, H]
    b2_r = np.asarray(fc2_b, np.float32).reshape(1, H)

    # L3 packings, per half
    f1T_half, b1_half, f2T_half = [], [], []
    for half in range(2):
        cols = slice(half * 2 * H, (half + 1) * 2 * H)
        f1h = fc1T[:, cols]                                         # [1024, 2048]
        # f1TD[g, p, 1024*(m%4) + 128c + j] = f1h[128c + p, 128m + j]
        a = (f1h.reshape(NHC, 128, 16, 128)      # [c, p, m, j]
             .transpose(2, 1, 0, 3)              # [m, p, c, j]
             .reshape(4, 4, 128, NHC * 128)      # [g, m%4, p, c*j]
             .transpose(0, 2, 1, 3)              # [g, p, m%4, c*j]
             .reshape(4, 128, 4 * H))
        f1T_half.append(np.ascontiguousarray(a.astype(NBF)))
        b1h = b1_eff[half * 2 * H:(half + 1) * 2 * H]
        b1_half.append(np.ascontiguousarray(
            b1h.reshape(16, 128).T).astype(np.float32))
        f2h = fc2T[half * 2 * H:(half + 1) * 2 * H, :]              # [2048, 1024]
        # f2TD[g, p, 1024*(m%4) + f] = f2h[128m + p, f]
        b = (f2h.reshape(4, 4, 128, H)           # [g, m%4, p, f]
             .transpose(0, 2, 1, 3)              # [g, p, m%4, f]
             .reshape(4, 128, 4 * H))
        f2T_half.append(np.ascontiguousarray(b.astype(NBF)))
    return dict(wQ=wQ_i, wK=wK_i, wV=wV_i, bcol=bcol_i, bvr=bvr_i, db=db_r,
                dwT=dwT, f1T_half=f1T_half, b1_half=b1_half,
                f2T_half=f2T_half, b2=b2_r)


def _tri_mask():
    k = np.arange(QB)[:, None]
    q = np.arange(QB)[None, :]
    return np.where(k <= q, 1.0, 0.0).astype(NBF)   # [k, q] allowed k<=q


def kernel(hidden_states, attention_mask, alibi,
           ln1_g, ln1_b, qkv_w, qkv_b, dense_w, dense_b,
           ln2_g, ln2_b, fc1_w, fc1_b, fc2_w, fc2_b):
    X = np.asarray(hidden_states, np.float32).reshape(S, H)
    alibi_np = np.asarray(alibi, np.float32).reshape(NH, S)
    W = _prep_weights(ln1_g, ln1_b, qkv_w, qkv_b, dense_w, dense_b,
                      ln2_g, ln2_b, fc1_w, fc1_b, fc2_w, fc2_b)

    # ---------------- L1: LN1 + transpose ----------------
    nc1 = _get_nc("l1")
    in1 = []
    for i in range(NCORE):
        a, b = _blocks(i)
        xi = np.concatenate([X[a * QB:(a + 1) * QB], X[b * QB:(b + 1) * QB]], 0)
        xp = np.ascontiguousarray(
            xi.reshape(2, 128, H).transpose(1, 0, 2)).astype(NBF)
        in1.append(dict(x=xp))
    r1 = _run(nc1, in1)

    # ---------------- host gather: xhat^T chunk-major ----------------
    xhG = np.zeros((NHC, 128, S), NBF)
    for i in range(NCORE):
        a, b = _blocks(i)
        r = r1[i]["xhT"].reshape(2, 128, NHC, 128)
        xhG[:, :, a * QB:(a + 1) * QB] = r[0].transpose(1, 0, 2)
        xhG[:, :, b * QB:(b + 1) * QB] = r[1].transpose(1, 0, 2)
    xhG = np.ascontiguousarray(xhG)

    # ---------------- L2: QKV + attention + dense partial ----------------
    binm = _tri_mask()
    dwT = W["dwT"]
    nc2 = _get_nc("l2")
    in2 = []
    for i in range(NCORE):
        alc = np.zeros((128, 32), np.float32)
        for j in range(2):
            alc[:, 16 * j:16 * (j + 1)] = \
                alibi_np[2 * i + j].reshape(16, 128).T
        dwi = np.ascontiguousarray(dwT[i * 128:(i + 1) * 128, :]).astype(NBF)
        in2.append(dict(xh=xhG, wQ=W["wQ"][i], wK=W["wK"][i], wV=W["wV"][i],
                        bw=W["bcol"][i], bvr=W["bvr"][i], alc=alc,
                        binm=binm, dw=dwi))
    r2 = _run(nc2, in2)

    # host reduce: attn_out = sum of dense partials + residual + dense bias
    attn_out = X + W["db"]
    for i in range(NCORE):
        attn_out = attn_out + r2[i]["outD"].astype(np.float32) \
            .reshape(128, 16, H).transpose(1, 0, 2).reshape(S, H)

    # ---------------- L3 ----------------
    nc3 = _get_nc("l3")
    in3 = []
    attn_bf = attn_out.astype(NBF)
    for i in range(NCORE):
        g, half = i // 2, i % 2
        xg = attn_bf[512 * g:512 * (g + 1)]        # [512, H]
        xres = np.ascontiguousarray(
            xg.reshape(4, 128, H).transpose(1, 0, 2).reshape(128, 4 * H))
        in3.append(dict(xres=xres, f1T=W["f1T_half"][half],
                        b1c=W["b1_half"][half], f2T=W["f2T_half"][half]))
    r3 = _run(nc3, in3)

    out = np.empty((S, H), np.float32)
    for g in range(4):
        p = r3[2 * g]["outP"].astype(np.float32) + \
            r3[2 * g + 1]["outP"].astype(np.float32)
        out[512 * g:512 * (g + 1)] = \
            p.reshape(128, 4, H).transpose(1, 0, 2).reshape(512, H) \
            + attn_out[512 * g:512 * (g + 1)] + W["b2"]
    return out.reshape(1, S, H)
